# revision 1
# baseline (speedup 1.0000x reference)
"""Causal self-attention (B=2, T=2048, D=2048, 16 heads) on 8 trn2 cores.

Sharding: tensor-parallel over heads — 2 heads per core. Each core computes
q/k/v projections for its 2 heads (column-parallel), causal attention per
head, and a partial output projection (row-parallel). Host sums the 8
partial outputs.

Layout strategy per core (all matmuls contract over the partition dim):
  xT   [D_MODEL, B*T]   (host-pretransposed x)
  wqT  [D_MODEL, 256]   (Wq rows for this core's heads, transposed)
  qT_h [128, B*T]       = WqT_h.T @ xT   (head dim on partitions)
  S.T  [j, i] tiles     = kT_tile.T @ qT_chunk    (keys on partitions)
  PT   = exp(S.T / sqrt(128)) * causal_mask       (ACT, PSUM->SBUF)
  den  [1, i]           = ones.T @ PT   (PE partition-sum, accumulated)
  outT [d, i]           = v_tile.T @ PT (accumulated over j tiles)
  normalize: outT *= broadcast(1/den)   (GpSimd bcast + DVE mult)
  y    [t, m] partial   = outT_tile.T @ WoT_chunk (accum over 2 heads)
"""

import math
from contextlib import ExitStack

import numpy as np
import ml_dtypes

import concourse.bass as bass
import concourse.mybir as mybir
import concourse.tile as tile
from concourse import bacc
from concourse.bass_utils import run_bass_kernel_spmd
from concourse.masks import make_identity

P = 128
D_MODEL = 2048
NUM_HEADS = 16
D = 128            # head dim
B, T = 2, 2048
BT = B * T         # 4096
NCORES = 8
HPC = NUM_HEADS // NCORES   # 2 heads per core
KD = D_MODEL // P           # 16 d_model tiles
TJ = T // P                 # 16 key tiles per batch
IC = 512                    # query chunk width
NI = T // IC                # 4 query chunks per batch
TCH = BT // IC              # 8 token chunks for projections

F32 = mybir.dt.float32

_DT = {
    "f32": mybir.dt.float32,
    "f32r": mybir.dt.float32r,
    "bf16": mybir.dt.bfloat16,
}
_NP = {
    "f32": np.float32,
    "f32r": np.float32,
    "bf16": ml_dtypes.bfloat16,
}

F32R = mybir.dt.float32r


# dtype config: x/w = projection inputs, s = qT/kT storage (S matmul inputs),
# pt = exp'd probabilities, v = value tiles, o = outT storage (outproj lhsT),
# wo = Wo tiles. overlap = double-buffer qkv arrays across heads (more SBUF).
CFG_SAFE = dict(x="f32", w="f32", s="f32", pt="f32", v="f32", o="f32",
                wo="f32", overlap=False)
CFG_FAST = dict(x="bf16", w="bf16", s="bf16", pt="bf16", v="bf16", o="f32r",
                wo="f32r", overlap=True)
# validated: rel err 2.27e-4 vs fp32 reference, ~650 us on HW
CFG_F32R = dict(x="f32r", w="f32r", s="f32r", pt="f32r", v="f32r", o="f32r",
                wo="f32r", overlap=False)


def _emit(tc, cfg, xT, wqT, wkT, wvT, woT, y):
    nc = tc.nc
    x_dt = _DT[cfg["x"]]
    w_dt = _DT[cfg["w"]]
    s_dt = _DT[cfg["s"]]
    pt_dt = _DT[cfg["pt"]]
    v_dt = _DT[cfg["v"]]
    o_dt = _DT[cfg["o"]]
    wo_dt = _DT[cfg["wo"]]
    qb = 2 if cfg["overlap"] else 1   # bufs for per-head qkv arrays

    with ExitStack() as ctx:
        consts = ctx.enter_context(tc.tile_pool(name="consts", bufs=1))
        wpool = ctx.enter_context(tc.tile_pool(name="wpool", bufs=2))
        xpool = ctx.enter_context(tc.tile_pool(name="xpool", bufs=7))
        wopool = ctx.enter_context(tc.tile_pool(name="wopool", bufs=4))
        arrs = ctx.enter_context(tc.tile_pool(name="arrs", bufs=qb))
        arrs2 = ctx.enter_context(tc.tile_pool(name="arrs2", bufs=2))
        ptpool = ctx.enter_context(tc.tile_pool(name="ptpool", bufs=5))
        smalls = ctx.enter_context(tc.tile_pool(name="smalls", bufs=2))
        ypool = ctx.enter_context(tc.tile_pool(name="ypool", bufs=2))
        psum = ctx.enter_context(tc.tile_pool(name="psum", bufs=1, space="PSUM"))
        def _const(shape, dt, tag, fill_fn):
            # gpsimd memset/affine can't write f32r; stage in f32 then copy.
            if dt == F32R:
                stg = consts.tile([P, IC], F32, tag="stg",
                                  name="stg")[:shape[0], :shape[1]]
                fill_fn(stg)
                out = consts.tile(shape, dt, tag=tag, name=tag)
                nc.vector.tensor_copy(out, stg)
                return out
            out = consts.tile(shape, dt, tag=tag, name=tag)
            fill_fn(out)
            return out

        ident = _const([P, P], v_dt, "ident", lambda t: make_identity(nc, t))
        ones_col = _const([P, 1], pt_dt, "ones",
                          lambda t: nc.vector.memset(t, 1.0))

        # tri_mask[p, i] = 1.0 if i >= p else 0 (upper triangular keep)
        def _fill_tri(t):
            nc.gpsimd.memset(t, 0.0)
            nc.gpsimd.affine_select(
                out=t, in_=t, compare_op=mybir.AluOpType.is_gt,
                fill=1.0, base=0, pattern=[[-1, P]], channel_multiplier=1,
            )

        tri_mask = _const([P, P], pt_dt, "trimask", _fill_tri)

        xT3 = xT.rearrange("(ko p) t -> p ko t", p=P)
        w3 = {
            "q": wqT.rearrange("(ko p) o -> p ko o", p=P),
            "k": wkT.rearrange("(ko p) o -> p ko o", p=P),
            "v": wvT.rearrange("(ko p) o -> p ko o", p=P),
        }

        outTs = []
        scale = 1.0 / math.sqrt(D)

        for h in range(HPC):
            # ---- projections for head h: qT/kT [128, BT], vT -> v ----
            w_sb = {}
            for nm in ("q", "k", "v"):
                wt = wpool.tile([P, KD, D], w_dt, tag=f"w{nm}")
                nc.sync.dma_start(wt, w3[nm][:, :, h * D:(h + 1) * D])
                w_sb[nm] = wt
            qT = arrs.tile([P, BT], s_dt, tag="qT")
            kT = arrs.tile([P, BT], s_dt, tag="kT")
            vT = arrs.tile([P, BT], v_dt, tag="vT")
            dests = {"q": qT, "k": kT, "v": vT}
            for tch in range(TCH):
                tsl = slice(tch * IC, (tch + 1) * IC)
                ps = {nm: psum.tile([P, IC], F32, tag=f"s{i}", name=f"s{i}")
                      for i, nm in enumerate(("q", "k", "v"))}
                for kt in range(KD):
                    xt = xpool.tile([P, IC], x_dt, tag="xt")
                    nc.sync.dma_start(xt, xT3[:, kt, tsl])
                    for nm in ("q", "k", "v"):
                        nc.tensor.matmul(
                            ps[nm], w_sb[nm][:, kt], xt,
                            start=(kt == 0), stop=(kt == KD - 1),
                        )
                for nm in ("q", "k", "v"):
                    nc.vector.tensor_copy(dests[nm][:, tsl], ps[nm])

            # transpose vT -> v [128, B, TJ, D] (token tiles on partitions)
            v_sb = arrs.tile([P, B, TJ, D], v_dt, tag="v")
            for b in range(B):
                for jt in range(TJ):
                    pst = psum.tile([P, P], v_dt, tag="s3")
                    nc.tensor.transpose(
                        pst, vT[:, b * T + jt * P: b * T + (jt + 1) * P], ident)
                    nc.vector.tensor_copy(v_sb[:, b, jt], pst)

            # ---- attention for head h ----
            outT = arrs2.tile([P, BT], o_dt, tag="outT")
            outTs.append(outT)
            for b in range(B):
                for ic in range(NI):
                    isl = slice(b * T + ic * IC, b * T + (ic + 1) * IC)
                    nj = ic * 4 + 4          # causal: j tiles 0..nj-1
                    ck = h * B * NI + b * NI + ic
                    ps_o = psum.tile([P, IC], F32, tag=f"o{ck % 2}",
                                     name=f"o{ck % 2}")
                    pt_acc = smalls.tile([P, IC], pt_dt, tag="ptacc")
                    for jt in range(nj):
                        m = jt - ic * 4
                        # partial diagonal tiles: columns < m*128 are fully
                        # masked; restrict all work to the live sub-range.
                        lo = max(m, 0) * P
                        ps_s = psum.tile([P, IC], F32, tag=f"s{jt % 4}",
                                         name=f"s{jt % 4}")
                        nc.tensor.matmul(
                            ps_s[:, lo:],
                            kT[:, b * T + jt * P: b * T + (jt + 1) * P],
                            qT[:, b * T + ic * IC + lo:
                               b * T + (ic + 1) * IC], start=True, stop=True,
                        )
                        pt = ptpool.tile([P, IC], pt_dt, tag="pt")
                        nc.scalar.activation(
                            pt[:, lo:], ps_s[:, lo:],
                            mybir.ActivationFunctionType.Exp, scale=scale)
                        if m >= 0:
                            nc.vector.tensor_tensor(
                                pt[:, lo:lo + P], pt[:, lo:lo + P],
                                tri_mask, mybir.AluOpType.mult)
                        if jt == 0:
                            nc.vector.tensor_copy(pt_acc, pt)
                        else:
                            nc.vector.tensor_tensor(
                                pt_acc[:, lo:], pt_acc[:, lo:], pt[:, lo:],
                                mybir.AluOpType.add)
                        nc.tensor.matmul(
                            ps_o[:, lo:], v_sb[:, b, jt], pt[:, lo:],
                            start=(jt == 0), stop=(jt == nj - 1),
                            skip_group_check=True,
                        )
                    # denominators: one partition-sum matmul per chunk
                    ps_d = psum.tile([1, IC], F32, tag="den", name="den")
                    nc.tensor.matmul(ps_d, ones_col, pt_acc,
                                     start=True, stop=True,
                                     skip_group_check=True)
                    den_sb = smalls.tile([1, IC], F32, tag="densb")
                    nc.vector.tensor_copy(den_sb, ps_d)
                    bc = smalls.tile([P, IC], F32, tag="bc")
                    nc.gpsimd.partition_broadcast(bc, den_sb)
                    rb = smalls.tile([P, IC], F32, tag="rb")
                    nc.vector.reciprocal_approx_fast(out=rb, in_=bc)
                    nc.vector.tensor_tensor(
                        outT[:, isl], ps_o, rb, mybir.AluOpType.mult)

        # ---- output projection: y[t, m] partial over this core's heads ----
        woT3 = woT.rearrange("(h p) m -> h p m", p=P)
        for mc in range(D_MODEL // IC):
            msl = slice(mc * IC, (mc + 1) * IC)
            wo_sb = []
            for h in range(HPC):
                wt = wopool.tile([P, IC], wo_dt, tag="wo", name="wo")
                nc.sync.dma_start(wt, woT3[h, :, msl])
                wo_sb.append(wt)
            for tt in range(BT // P):
                ps_y = psum.tile([P, IC], F32, tag="y")
                for h in range(HPC):
                    nc.tensor.matmul(
                        ps_y, outTs[h][:, tt * P:(tt + 1) * P], wo_sb[h],
                        start=(h == 0), stop=(h == HPC - 1),
                    )
                y_sb = ypool.tile([P, IC], F32, tag="y")
                if mc == 0:
                    nc.vector.tensor_copy(y_sb, ps_y)
                else:
                    nc.scalar.copy(y_sb, ps_y)
                nc.sync.dma_start(y[tt * P:(tt + 1) * P, msl], y_sb)


def _build(cfg):
    nc = bacc.Bacc("TRN2", target_bir_lowering=False, debug=False,
                   num_devices=NCORES)
    xT = nc.dram_tensor("xT", [D_MODEL, BT], _DT[cfg["x"]],
                        kind="ExternalInput").ap()
    wqT = nc.dram_tensor("wqT", [D_MODEL, HPC * D], _DT[cfg["w"]],
                         kind="ExternalInput").ap()
    wkT = nc.dram_tensor("wkT", [D_MODEL, HPC * D], _DT[cfg["w"]],
                         kind="ExternalInput").ap()
    wvT = nc.dram_tensor("wvT", [D_MODEL, HPC * D], _DT[cfg["w"]],
                         kind="ExternalInput").ap()
    woT = nc.dram_tensor("woT", [HPC * D, D_MODEL], _DT[cfg["wo"]],
                         kind="ExternalInput").ap()
    y = nc.dram_tensor("y", [BT, D_MODEL], F32, kind="ExternalOutput").ap()
    with tile.TileContext(nc) as tc:
        _emit(tc, cfg, xT, wqT, wkT, wvT, woT, y)
    nc.compile()
    return nc


def _prep_inputs(x, Wq, Wk, Wv, Wo, cfg):
    xnp = _NP[cfg["x"]]
    wnp = _NP[cfg["w"]]
    wonp = _NP[cfg["wo"]]
    xT = np.ascontiguousarray(
        np.asarray(x, np.float32).reshape(BT, D_MODEL).T).astype(xnp)
    in_maps = []
    for c in range(NCORES):
        rows = slice(c * HPC * D, (c + 1) * HPC * D)
        in_maps.append({
            "xT": xT,
            "wqT": np.ascontiguousarray(np.asarray(Wq)[rows].T).astype(wnp),
            "wkT": np.ascontiguousarray(np.asarray(Wk)[rows].T).astype(wnp),
            "wvT": np.ascontiguousarray(np.asarray(Wv)[rows].T).astype(wnp),
            "woT": np.ascontiguousarray(
                np.asarray(Wo)[:, rows].T).astype(wonp),
        })
    return in_maps


def run(x, Wq, Wk, Wv, Wo, cfg=None, trace=False):
    cfg = cfg or CFG_F32R
    nc = _build(cfg)
    in_maps = _prep_inputs(x, Wq, Wk, Wv, Wo, cfg)
    try:
        res = run_bass_kernel_spmd(nc, in_maps, core_ids=list(range(NCORES)),
                                   trace=trace)
    except Exception:
        res = run_bass_kernel_spmd(nc, in_maps, core_ids=list(range(NCORES)),
                                   trace=trace)
    y = np.zeros((BT, D_MODEL), np.float32)
    for r in res.results:
        y += r["y"]
    return y.reshape(B, T, D_MODEL), res


def kernel(x, Wq, Wk, Wv, Wo):
    y, _ = run(x, Wq, Wk, Wv, Wo)
    return y



# revision 8
# speedup vs baseline: 1.4372x; 1.4372x over previous
"""Causal self-attention (B=2, T=2048, D=2048, 16 heads) on 8 trn2 cores.

Sharding: tensor-parallel over heads — 2 heads per core. Each core computes
q/k/v projections for its 2 heads (column-parallel), causal attention per
head, and a partial output projection (row-parallel). Host sums the 8
partial outputs.

v2 layout (all matmul inputs bf16, PSUM accumulation f32):
  xT   [D_MODEL, B*T] bf16, streamed ONCE per core in [128, 16, 512] chunks
  per chunk: 6 accumulations (2 heads x q/k/v) reuse the resident x tiles
  qT/kT/vT [128, B*T] bf16 per head (head dim on partitions)
  v_sb transposed via PE (token tiles on partitions), interleaved with
  projections to keep the PE dense
  attention per (head, batch, 512-query chunk): S^T tiles -> exp (ACT,
  bf16) -> causal mask (DVE) -> PV accumulation; denominator via
  ones^T @ pt_acc partition-sum matmul; normalize with gpsimd broadcast +
  DVE multiply
  y    [B*T, D_MODEL] fp16 partial = outT_tile.T @ Wo chunk (accum 2 heads)
"""

import math
from contextlib import ExitStack

import numpy as np
import ml_dtypes

import concourse.bass as bass
import concourse.mybir as mybir
import concourse.tile as tile
from concourse import bacc
from concourse.bass_isa import ReduceOp
from concourse.bass_utils import run_bass_kernel_spmd
from concourse.masks import make_identity

P = 128
D_MODEL = 2048
NUM_HEADS = 16
D = 128            # head dim
B, T = 2, 2048
BT = B * T         # 4096
NCORES = 8
HPC = NUM_HEADS // NCORES   # 2 heads per core
KD = D_MODEL // P           # 16 d_model tiles
TJ = T // P                 # 16 key tiles per batch
IC = 512                    # query chunk width
NI = T // IC                # 4 query chunks per batch
TCH = BT // IC              # 8 token chunks for projections

F32 = mybir.dt.float32
BF16 = mybir.dt.bfloat16
F16 = mybir.dt.float16

# kept for test.py compat; v2 is bf16-everywhere and ignores these knobs
CFG_SAFE = dict()
CFG_FAST = dict()
CFG_F32R = dict()


def _emit(tc, xT, wqT, wkT, wvT, woT, y):
    nc = tc.nc

    with ExitStack() as ctx:
        consts = ctx.enter_context(tc.tile_pool(name="consts", bufs=1))
        wpool = ctx.enter_context(tc.tile_pool(name="wpool", bufs=1))
        xpool = ctx.enter_context(tc.tile_pool(name="xpool", bufs=2))
        arrs = ctx.enter_context(tc.tile_pool(name="arrs", bufs=1))
        ptpool = ctx.enter_context(tc.tile_pool(name="ptpool", bufs=6))
        smalls = ctx.enter_context(tc.tile_pool(name="smalls", bufs=2))
        ypool = ctx.enter_context(tc.tile_pool(name="ypool", bufs=4))
        psum = ctx.enter_context(tc.tile_pool(name="psum", bufs=1, space="PSUM"))

        ident = consts.tile([P, P], BF16, tag="ident", name="ident")
        make_identity(nc, ident)

        # tri_mask[p, i] = 1.0 if i >= p else 0 (upper triangular keep)
        tri_mask = consts.tile([P, P], BF16, tag="trimask", name="trimask")
        nc.gpsimd.memset(tri_mask, 0.0)
        nc.gpsimd.affine_select(
            out=tri_mask, in_=tri_mask, compare_op=mybir.AluOpType.is_gt,
            fill=1.0, base=0, pattern=[[-1, P]], channel_multiplier=1,
        )

        xT3 = xT.rearrange("(ko p) t -> p ko t", p=P)
        w3 = {
            "q": wqT.rearrange("(ko p) (h d) -> p ko h d", p=P, h=HPC),
            "k": wkT.rearrange("(ko p) (h d) -> p ko h d", p=P, h=HPC),
            "v": wvT.rearrange("(ko p) (h d) -> p ko h d", p=P, h=HPC),
        }
        woT3 = woT.rearrange("(h p) m -> p h m", p=P)

        # resident weights: 3 x [128, KD, HPC, 128] bf16 (8KB/part each)
        w_sb = {}
        for nm in ("q", "k", "v"):
            wt = wpool.tile([P, KD, HPC, D], BF16, tag=f"w{nm}", name=f"w{nm}")
            nc.sync.dma_start(wt, w3[nm])
            w_sb[nm] = wt
        wo_sb = wpool.tile([P, HPC, D_MODEL], BF16, tag="wo", name="wo")
        nc.sync.dma_start(wo_sb, woT3)

        # per-head arrays
        qT = [arrs.tile([P, BT], BF16, tag=f"qT{h}", name=f"qT{h}")
              for h in range(HPC)]
        kT = [arrs.tile([P, BT], BF16, tag=f"kT{h}", name=f"kT{h}")
              for h in range(HPC)]
        vT = [arrs.tile([P, BT], BF16, tag=f"vT{h}", name=f"vT{h}")
              for h in range(HPC)]
        v_sb = [arrs.tile([P, B * TJ, D], BF16, tag=f"v{h}", name=f"v{h}")
                for h in range(HPC)]
        outT = [arrs.tile([P, BT], BF16, tag=f"outT{h}", name=f"outT{h}")
                for h in range(HPC)]
        # transpose staging: 2 rotating [P, P] bf16 sub-buffers in PSUM
        tp = psum.tile([P, 2, P], BF16, tag="tp", name="tp")

        # ---- projections: one pass over x, 6 accumulations per chunk ----
        for tch in range(TCH):
            tsl = slice(tch * IC, (tch + 1) * IC)
            xt = xpool.tile([P, KD, IC], BF16, tag="xt")
            nc.sync.dma_start(xt, xT3[:, :, tsl])
            for h in range(HPC):
                ps = {nm: psum.tile([P, IC], F32, tag=f"p{i}", name=f"p{i}")
                      for i, nm in enumerate(("q", "k", "v"))}
                for kt in range(KD):
                    for nm in ("q", "k", "v"):
                        nc.tensor.matmul(
                            ps[nm], w_sb[nm][:, kt, h], xt[:, kt],
                            start=(kt == 0), stop=(kt == KD - 1),
                        )
                nc.scalar.copy(qT[h][:, tsl], ps["q"])
                nc.scalar.copy(kT[h][:, tsl], ps["k"])
                nc.vector.tensor_copy(vT[h][:, tsl], ps["v"])
                # transpose this chunk's v tiles now (keeps PE dense)
                for i in range(IC // P):
                    tt0 = tch * (IC // P) + i   # global token tile 0..31
                    g = (tch * HPC + h) * (IC // P) + i
                    nc.tensor.transpose(
                        tp[:, g % 2], vT[h][:, tt0 * P:(tt0 + 1) * P], ident)
                    nc.vector.tensor_copy(v_sb[h][:, tt0], tp[:, g % 2])

        # ---- attention ----
        scale = 1.0 / math.sqrt(D)
        for h in range(HPC):
            for b in range(B):
                for ic in range(NI):
                    isl = slice(b * T + ic * IC, b * T + (ic + 1) * IC)
                    nj = ic * 4 + 4          # causal: j tiles 0..nj-1
                    ck = h * B * NI + b * NI + ic
                    ps_o = psum.tile([P, IC], F32, tag=f"t{ck % 2}",
                                     name=f"t{ck % 2}")
                    pt_acc = smalls.tile([P, IC], BF16, tag="ptacc")
                    for jt in range(nj):
                        m = jt - ic * 4
                        # partial diagonal tiles: columns < m*128 are fully
                        # masked; restrict all work to the live sub-range.
                        lo = max(m, 0) * P
                        ps_s = psum.tile([P, IC], F32, tag=f"p{jt % 3}",
                                         name=f"p{jt % 3}")
                        nc.tensor.matmul(
                            ps_s[:, lo:],
                            kT[h][:, b * T + jt * P: b * T + (jt + 1) * P],
                            qT[h][:, b * T + ic * IC + lo:
                                  b * T + (ic + 1) * IC],
                            start=True, stop=True,
                        )
                        pt = ptpool.tile([P, IC], BF16, tag="pt")
                        nc.scalar.activation(
                            pt[:, lo:], ps_s[:, lo:],
                            mybir.ActivationFunctionType.Exp, scale=scale)
                        if m >= 0:
                            nc.vector.tensor_tensor(
                                pt[:, lo:lo + P], pt[:, lo:lo + P],
                                tri_mask, mybir.AluOpType.mult)
                        if jt == 0:
                            nc.vector.tensor_copy(pt_acc, pt)
                        else:
                            nc.vector.tensor_tensor(
                                pt_acc[:, lo:], pt_acc[:, lo:], pt[:, lo:],
                                mybir.AluOpType.add)
                        nc.tensor.matmul(
                            ps_o[:, lo:], v_sb[h][:, b * TJ + jt],
                            pt[:, lo:],
                            start=(jt == 0), stop=(jt == nj - 1),
                            skip_group_check=True,
                        )
                    # denominator: gpsimd all-reduce across partitions
                    # (result broadcast to all 128 partitions)
                    bc = smalls.tile([P, IC], F32, tag="bc")
                    nc.gpsimd.partition_all_reduce(bc, pt_acc, P,
                                                   ReduceOp.add)
                    rb = smalls.tile([P, IC], F32, tag="rb")
                    nc.vector.reciprocal_approx_fast(out=rb, in_=bc)
                    nc.vector.tensor_tensor(
                        outT[h][:, isl], ps_o, rb, mybir.AluOpType.mult)

        # ---- output projection: y[t, m] partial over this core's heads ----
        for mc in range(D_MODEL // IC):
            msl = slice(mc * IC, (mc + 1) * IC)
            for tt in range(BT // P):
                ps_y = psum.tile([P, IC], F32, tag=f"y{tt % 2}",
                                 name=f"y{tt % 2}")
                for h in range(HPC):
                    nc.tensor.matmul(
                        ps_y, outT[h][:, tt * P:(tt + 1) * P],
                        wo_sb[:, h, msl],
                        start=(h == 0), stop=(h == HPC - 1),
                    )
                y_sb = ypool.tile([P, IC], F16, tag="y")
                if tt % 2:
                    nc.scalar.copy(y_sb, ps_y)
                else:
                    nc.vector.tensor_copy(y_sb, ps_y)
                nc.sync.dma_start(y[tt * P:(tt + 1) * P, msl], y_sb)


def _build(cfg=None):
    nc = bacc.Bacc("TRN2", target_bir_lowering=False, debug=False,
                   num_devices=NCORES)
    xT = nc.dram_tensor("xT", [D_MODEL, BT], BF16, kind="ExternalInput").ap()
    wqT = nc.dram_tensor("wqT", [D_MODEL, HPC * D], BF16,
                         kind="ExternalInput").ap()
    wkT = nc.dram_tensor("wkT", [D_MODEL, HPC * D], BF16,
                         kind="ExternalInput").ap()
    wvT = nc.dram_tensor("wvT", [D_MODEL, HPC * D], BF16,
                         kind="ExternalInput").ap()
    woT = nc.dram_tensor("woT", [HPC * D, D_MODEL], BF16,
                         kind="ExternalInput").ap()
    y = nc.dram_tensor("y", [BT, D_MODEL], F16, kind="ExternalOutput").ap()
    with tile.TileContext(nc) as tc:
        _emit(tc, xT, wqT, wkT, wvT, woT, y)
    nc.compile()
    return nc


def _prep_inputs(x, Wq, Wk, Wv, Wo, cfg=None):
    bf = ml_dtypes.bfloat16
    xT = np.ascontiguousarray(
        np.asarray(x, np.float32).reshape(BT, D_MODEL).T).astype(bf)
    in_maps = []
    for c in range(NCORES):
        rows = slice(c * HPC * D, (c + 1) * HPC * D)
        in_maps.append({
            "xT": xT,
            "wqT": np.ascontiguousarray(np.asarray(Wq)[rows].T).astype(bf),
            "wkT": np.ascontiguousarray(np.asarray(Wk)[rows].T).astype(bf),
            "wvT": np.ascontiguousarray(np.asarray(Wv)[rows].T).astype(bf),
            "woT": np.ascontiguousarray(np.asarray(Wo)[:, rows].T).astype(bf),
        })
    return in_maps


def run(x, Wq, Wk, Wv, Wo, cfg=None, trace=False):
    nc = _build(cfg)
    in_maps = _prep_inputs(x, Wq, Wk, Wv, Wo, cfg)
    try:
        res = run_bass_kernel_spmd(nc, in_maps, core_ids=list(range(NCORES)),
                                   trace=trace)
    except Exception:
        res = run_bass_kernel_spmd(nc, in_maps, core_ids=list(range(NCORES)),
                                   trace=trace)
    y = np.zeros((BT, D_MODEL), np.float32)
    for r in res.results:
        y += np.asarray(r["y"], np.float32)
    return y.reshape(B, T, D_MODEL), res


def kernel(x, Wq, Wk, Wv, Wo):
    y, _ = run(x, Wq, Wk, Wv, Wo)
    return y


# revision 9
# speedup vs baseline: 1.6022x; 1.1148x over previous
"""Causal self-attention (B=2, T=2048, D=2048, 16 heads) on 8 trn2 cores.

Sharding: tensor-parallel over heads — 2 heads per core. Each core computes
q/k/v projections for its 2 heads (column-parallel), causal attention per
head, and a partial output projection (row-parallel). Host sums the 8
partial outputs.

v3: phase-interleaved schedule to keep the PE dense (HAM warm) and hide
the scalar-engine exp under projection matmuls:
  A: head-0 q/k/v projections (+ v transposes via PE)
  B: head-1 projections interleaved with head-0 attention chunks
  C: head-1 attention interleaved with out-projection of the previous
     chunk's token range (norm-chain latency hidden)
Causal mask applied on the PE: diagonal S tiles accumulate a constant
strictly-lower -1000 bias via a second matmul, so exp underflows to 0 and
no vector-engine op sits between exp and the PV matmul. Softmax
denominator via gpsimd partition_all_reduce, off the critical path
(ps_o is freed by an unnormalized copy; normalize happens later on DVE).
All matmul inputs bf16, PSUM f32, output partial y in fp16.
"""

import math
from contextlib import ExitStack

import numpy as np
import ml_dtypes

import concourse.bass as bass
import concourse.mybir as mybir
import concourse.tile as tile
from concourse import bacc
from concourse.bass_isa import ReduceOp
from concourse.bass_utils import run_bass_kernel_spmd
from concourse.masks import make_identity

P = 128
D_MODEL = 2048
NUM_HEADS = 16
D = 128            # head dim
B, T = 2, 2048
BT = B * T         # 4096
NCORES = 8
HPC = NUM_HEADS // NCORES   # 2 heads per core
KD = D_MODEL // P           # 16 d_model tiles
TJ = T // P                 # 16 key tiles per batch
IC = 512                    # query chunk width
NI = T // IC                # 4 query chunks per batch
TCH = BT // IC              # 8 token chunks for projections

F32 = mybir.dt.float32
BF16 = mybir.dt.bfloat16
F16 = mybir.dt.float16

# kept for test.py compat; v3 is bf16-everywhere and ignores these knobs
CFG_SAFE = dict()
CFG_FAST = dict()
CFG_F32R = dict()


class _Ctx:
    pass


def _emit(tc, xT, wqT, wkT, wvT, woT, y):
    nc = tc.nc
    scale = 1.0 / math.sqrt(D)

    with ExitStack() as ctx:
        consts = ctx.enter_context(tc.tile_pool(name="consts", bufs=1))
        wpool = ctx.enter_context(tc.tile_pool(name="wpool", bufs=1))
        xpool = ctx.enter_context(tc.tile_pool(name="xpool", bufs=2))
        arrs = ctx.enter_context(tc.tile_pool(name="arrs", bufs=1))
        ptpool = ctx.enter_context(tc.tile_pool(name="ptpool", bufs=6))
        smalls = ctx.enter_context(tc.tile_pool(name="smalls", bufs=2))
        ypool = ctx.enter_context(tc.tile_pool(name="ypool", bufs=4))
        psum = ctx.enter_context(tc.tile_pool(name="psum", bufs=1, space="PSUM"))

        ident = consts.tile([P, P], BF16, tag="ident", name="ident")
        make_identity(nc, ident)

        # tri_mask[p, i] = 1.0 if i >= p else 0 (upper triangular keep)
        tri_mask = consts.tile([P, P], BF16, tag="trimask", name="trimask")
        nc.gpsimd.memset(tri_mask, 0.0)
        nc.gpsimd.affine_select(
            out=tri_mask, in_=tri_mask, compare_op=mybir.AluOpType.is_gt,
            fill=1.0, base=0, pattern=[[-1, P]], channel_multiplier=1,
        )
        # triC[i, p] = -1000 where p > i else 0  (strictly upper).
        # Used as lhsT in a bias matmul: (triC.T @ I)[p, i] = -1000 for p > i,
        # i.e. key-row p beyond query-col i -> exp underflows to 0.
        triC = consts.tile([P, P], BF16, tag="triC", name="triC")
        nc.vector.tensor_tensor(triC, tri_mask, ident,
                                mybir.AluOpType.subtract)
        nc.vector.tensor_scalar_mul(triC, triC, -1000.0)

        xT3 = xT.rearrange("(ko p) t -> p ko t", p=P)
        w3 = {
            "q": wqT.rearrange("(ko p) (h d) -> p ko h d", p=P, h=HPC),
            "k": wkT.rearrange("(ko p) (h d) -> p ko h d", p=P, h=HPC),
            "v": wvT.rearrange("(ko p) (h d) -> p ko h d", p=P, h=HPC),
        }
        woT3 = woT.rearrange("(h p) m -> p h m", p=P)

        # resident weights
        w_sb = {}
        for nm in ("q", "k", "v"):
            wt = wpool.tile([P, KD, HPC, D], BF16, tag=f"w{nm}", name=f"w{nm}")
            nc.sync.dma_start(wt, w3[nm])
            w_sb[nm] = wt
        wo_sb = wpool.tile([P, HPC, D_MODEL], BF16, tag="wo", name="wo")
        nc.sync.dma_start(wo_sb, woT3)

        # per-head arrays
        qT = [arrs.tile([P, BT], BF16, tag=f"qT{h}", name=f"qT{h}")
              for h in range(HPC)]
        kT = [arrs.tile([P, BT], BF16, tag=f"kT{h}", name=f"kT{h}")
              for h in range(HPC)]
        vT = [arrs.tile([P, BT], BF16, tag=f"vT{h}", name=f"vT{h}")
              for h in range(HPC)]
        v_sb = [arrs.tile([P, B * TJ, D], BF16, tag=f"v{h}", name=f"v{h}")
                for h in range(HPC)]
        outT = [arrs.tile([P, BT], BF16, tag=f"outT{h}", name=f"outT{h}")
                for h in range(HPC)]
        # transpose staging: 2 rotating [P, P] bf16 sub-buffers in PSUM
        tp = psum.tile([P, 2, P], BF16, tag="tp", name="tp")

        st = _Ctx()
        st.tpg = 0     # transpose rotation counter
        st.yc = 0      # outproj psum rotation counter

        def emit_proj_tch(h, tch):
            """q/k/v projection + v transposes for one 512-token chunk."""
            tsl = slice(tch * IC, (tch + 1) * IC)
            xt = xpool.tile([P, KD, IC], BF16, tag="xt", name="xt")
            nc.sync.dma_start(xt, xT3[:, :, tsl])
            ps = {nm: psum.tile([P, IC], F32, tag=f"p{i}", name=f"p{i}")
                  for i, nm in enumerate(("q", "k", "v"))}
            for kt in range(KD):
                for nm in ("q", "k", "v"):
                    nc.tensor.matmul(
                        ps[nm], w_sb[nm][:, kt, h], xt[:, kt],
                        start=(kt == 0), stop=(kt == KD - 1),
                    )
            nc.scalar.copy(qT[h][:, tsl], ps["q"])
            nc.scalar.copy(kT[h][:, tsl], ps["k"])
            nc.vector.tensor_copy(vT[h][:, tsl], ps["v"])
            for i in range(IC // P):
                tt0 = tch * (IC // P) + i   # global token tile 0..31
                g = st.tpg
                st.tpg += 1
                nc.tensor.transpose(
                    tp[:, g % 2], vT[h][:, tt0 * P:(tt0 + 1) * P], ident)
                nc.vector.tensor_copy(v_sb[h][:, tt0], tp[:, g % 2])

        def emit_attn_chunk(h, b, ic, srot, ostag):
            """Attention for one 512-query chunk of head h."""
            isl = slice(b * T + ic * IC, b * T + (ic + 1) * IC)
            nj = ic * 4 + 4          # causal: j tiles 0..nj-1
            ps_o = psum.tile([P, IC], F32, tag=ostag, name=ostag)
            pt_acc = smalls.tile([P, IC], BF16, tag="ptacc")
            for jt in range(nj):
                m = jt - ic * 4
                # partial diagonal tiles: columns < m*128 are fully masked
                lo = max(m, 0) * P
                tag = srot[jt % len(srot)]
                ps_s = psum.tile([P, IC], F32, tag=tag, name=tag)
                if m >= 0:
                    nc.tensor.matmul(
                        ps_s[:, lo:],
                        kT[h][:, b * T + jt * P: b * T + (jt + 1) * P],
                        qT[h][:, b * T + ic * IC + lo:
                              b * T + (ic + 1) * IC],
                        start=True, stop=False,
                    )
                    nc.tensor.matmul(
                        ps_s[:, lo:lo + P], triC, ident,
                        start=False, stop=True, skip_group_check=True,
                    )
                else:
                    nc.tensor.matmul(
                        ps_s[:, lo:],
                        kT[h][:, b * T + jt * P: b * T + (jt + 1) * P],
                        qT[h][:, b * T + ic * IC + lo:
                              b * T + (ic + 1) * IC],
                        start=True, stop=True,
                    )
                pt = ptpool.tile([P, IC], BF16, tag="pt", name="pt")
                nc.scalar.activation(
                    pt[:, lo:], ps_s[:, lo:],
                    mybir.ActivationFunctionType.Exp, scale=scale)
                nc.tensor.matmul(
                    ps_o[:, lo:], v_sb[h][:, b * TJ + jt], pt[:, lo:],
                    start=(jt == 0), stop=(jt == nj - 1),
                    skip_group_check=True,
                )
                # denominator accumulation, off the PE critical path
                if jt == 0:
                    nc.vector.tensor_copy(pt_acc, pt)
                else:
                    nc.vector.tensor_tensor(
                        pt_acc[:, lo:], pt_acc[:, lo:], pt[:, lo:],
                        mybir.AluOpType.add)
            # free ps_o fast with an unnormalized copy; normalize later
            o_u = smalls.tile([P, IC], BF16, tag="ou")
            nc.scalar.copy(o_u, ps_o)
            bc = smalls.tile([P, IC], F32, tag="bc")
            nc.gpsimd.partition_all_reduce(bc, pt_acc, P, ReduceOp.add)
            rb = smalls.tile([P, IC], F32, tag="rb")
            nc.vector.reciprocal_approx_fast(out=rb, in_=bc)
            nc.vector.tensor_tensor(
                outT[h][:, isl], o_u, rb, mybir.AluOpType.mult)

        def emit_outproj_chunk(b, ic):
            """y tiles (all 2048 out-channels) for one 512-token range."""
            t0 = (b * T + ic * IC) // P
            for mc in range(D_MODEL // IC):
                msl = slice(mc * IC, (mc + 1) * IC)
                for tl in range(IC // P):
                    tt = t0 + tl
                    yc = st.yc
                    st.yc += 1
                    ps_y = psum.tile([P, IC], F32, tag=f"y{yc % 2}",
                                     name=f"y{yc % 2}")
                    for h in range(HPC):
                        nc.tensor.matmul(
                            ps_y, outT[h][:, tt * P:(tt + 1) * P],
                            wo_sb[:, h, msl],
                            start=(h == 0), stop=(h == HPC - 1),
                        )
                    y_sb = ypool.tile([P, IC], F16, tag="y", name="y")
                    if yc % 2:
                        nc.scalar.copy(y_sb, ps_y)
                    else:
                        nc.vector.tensor_copy(y_sb, ps_y)
                    nc.sync.dma_start(y[tt * P:(tt + 1) * P, msl], y_sb)

        # ---- phase A: head-0 projections ----
        for tch in range(TCH):
            emit_proj_tch(0, tch)

        # ---- phase B: head-1 projections x head-0 attention ----
        h0_chunks = [(bb, ii) for bb in range(B) for ii in range(NI)]
        for tch in range(TCH):
            emit_proj_tch(1, tch)
            bb, ii = h0_chunks[tch]
            emit_attn_chunk(0, bb, ii, srot=("y0", "y1", "t1"), ostag="t0")

        # ---- phase C: head-1 attention x out-projection (1 chunk behind) ----
        prev = None
        for bb, ii in h0_chunks:
            emit_attn_chunk(1, bb, ii, srot=("p0", "p1", "p2"), ostag="t1")
            if prev is not None:
                emit_outproj_chunk(*prev)
            prev = (bb, ii)
        emit_outproj_chunk(*prev)


def _build(cfg=None):
    nc = bacc.Bacc("TRN2", target_bir_lowering=False, debug=False,
                   num_devices=NCORES)
    xT = nc.dram_tensor("xT", [D_MODEL, BT], BF16, kind="ExternalInput").ap()
    wqT = nc.dram_tensor("wqT", [D_MODEL, HPC * D], BF16,
                         kind="ExternalInput").ap()
    wkT = nc.dram_tensor("wkT", [D_MODEL, HPC * D], BF16,
                         kind="ExternalInput").ap()
    wvT = nc.dram_tensor("wvT", [D_MODEL, HPC * D], BF16,
                         kind="ExternalInput").ap()
    woT = nc.dram_tensor("woT", [HPC * D, D_MODEL], BF16,
                         kind="ExternalInput").ap()
    y = nc.dram_tensor("y", [BT, D_MODEL], F16, kind="ExternalOutput").ap()
    with tile.TileContext(nc) as tc:
        _emit(tc, xT, wqT, wkT, wvT, woT, y)
    nc.compile()
    return nc


def _prep_inputs(x, Wq, Wk, Wv, Wo, cfg=None):
    bf = ml_dtypes.bfloat16
    xT = np.ascontiguousarray(
        np.asarray(x, np.float32).reshape(BT, D_MODEL).T).astype(bf)
    in_maps = []
    for c in range(NCORES):
        rows = slice(c * HPC * D, (c + 1) * HPC * D)
        in_maps.append({
            "xT": xT,
            "wqT": np.ascontiguousarray(np.asarray(Wq)[rows].T).astype(bf),
            "wkT": np.ascontiguousarray(np.asarray(Wk)[rows].T).astype(bf),
            "wvT": np.ascontiguousarray(np.asarray(Wv)[rows].T).astype(bf),
            "woT": np.ascontiguousarray(np.asarray(Wo)[:, rows].T).astype(bf),
        })
    return in_maps


def run(x, Wq, Wk, Wv, Wo, cfg=None, trace=False):
    nc = _build(cfg)
    in_maps = _prep_inputs(x, Wq, Wk, Wv, Wo, cfg)
    try:
        res = run_bass_kernel_spmd(nc, in_maps, core_ids=list(range(NCORES)),
                                   trace=trace)
    except Exception:
        res = run_bass_kernel_spmd(nc, in_maps, core_ids=list(range(NCORES)),
                                   trace=trace)
    y = np.zeros((BT, D_MODEL), np.float32)
    for r in res.results:
        y += np.asarray(r["y"], np.float32)
    return y.reshape(B, T, D_MODEL), res


def kernel(x, Wq, Wk, Wv, Wo):
    y, _ = run(x, Wq, Wk, Wv, Wo)
    return y


# revision 10
# speedup vs baseline: 1.6469x; 1.0279x over previous
"""Causal self-attention (B=2, T=2048, D=2048, 16 heads) on 8 trn2 cores.

Sharding: tensor-parallel over heads — 2 heads per core. Each core computes
q/k/v projections for its 2 heads (column-parallel), causal attention per
head, and a partial output projection (row-parallel). Host sums the 8
partial outputs.

v4: generator-based fine-grained interleave so the PE never waits on the
scalar-engine exp:
  A: head-0 q/k/v projections (+ v transposes via PE)
  B: head-1 projection matmuls interleaved per-k-tile with head-0
     attention tiles (S issued 2 tiles ahead of PV)
  C: head-1 attention interleaved per-tile with out-projection of the
     token range two chunks back (hides the gpsimd denominator chain)
Causal mask applied on the PE: diagonal S tiles accumulate a constant
strictly-lower -1000 bias via a second matmul, so exp underflows to 0 and
no vector-engine op sits between exp and the PV matmul. Softmax
denominator via gpsimd partition_all_reduce, off the critical path
(ps_o is freed by an unnormalized copy; normalize happens later on DVE).
All matmul inputs bf16, PSUM f32, output partial y in fp16.
"""

import math
from contextlib import ExitStack

import numpy as np
import ml_dtypes

import concourse.bass as bass
import concourse.mybir as mybir
import concourse.tile as tile
from concourse import bacc
from concourse.bass_isa import ReduceOp
from concourse.bass_utils import run_bass_kernel_spmd
from concourse.masks import make_identity

P = 128
D_MODEL = 2048
NUM_HEADS = 16
D = 128            # head dim
B, T = 2, 2048
BT = B * T         # 4096
NCORES = 8
HPC = NUM_HEADS // NCORES   # 2 heads per core
KD = D_MODEL // P           # 16 d_model tiles
TJ = T // P                 # 16 key tiles per batch
IC = 512                    # query chunk width
NI = T // IC                # 4 query chunks per batch
TCH = BT // IC              # 8 token chunks for projections
LA = 2                      # S-tile lookahead ahead of PV consumption

F32 = mybir.dt.float32
BF16 = mybir.dt.bfloat16
F16 = mybir.dt.float16

# kept for test.py compat; v4 is bf16-everywhere and ignores these knobs
CFG_SAFE = dict()
CFG_FAST = dict()
CFG_F32R = dict()


class _Ctx:
    pass


def _run_gen(g):
    for _ in g:
        pass


def _drive(g_part, n_part, g_att, n_att):
    """Interleave partner units with attention units (Bresenham spread),
    attention first within each step; drain both at the end."""
    done = 0
    for s in range(n_part):
        want = ((s + 1) * n_att) // n_part
        while done < want:
            next(g_att, None)
            done += 1
        next(g_part, None)
    _run_gen(g_part)
    _run_gen(g_att)


def _emit(tc, xT, wqT, wkT, wvT, woT, y):
    nc = tc.nc
    scale = 1.0 / math.sqrt(D)

    with ExitStack() as ctx:
        consts = ctx.enter_context(tc.tile_pool(name="consts", bufs=1))
        wpool = ctx.enter_context(tc.tile_pool(name="wpool", bufs=1))
        xpool = ctx.enter_context(tc.tile_pool(name="xpool", bufs=2))
        arrs = ctx.enter_context(tc.tile_pool(name="arrs", bufs=1))
        ptpool = ctx.enter_context(tc.tile_pool(name="ptpool", bufs=6))
        smalls = ctx.enter_context(tc.tile_pool(name="smalls", bufs=2))
        ypool = ctx.enter_context(tc.tile_pool(name="ypool", bufs=4))
        psum = ctx.enter_context(tc.tile_pool(name="psum", bufs=1, space="PSUM"))

        ident = consts.tile([P, P], BF16, tag="ident", name="ident")
        make_identity(nc, ident)

        # tri_mask[p, i] = 1.0 if i >= p else 0 (upper triangular keep)
        tri_mask = consts.tile([P, P], BF16, tag="trimask", name="trimask")
        nc.gpsimd.memset(tri_mask, 0.0)
        nc.gpsimd.affine_select(
            out=tri_mask, in_=tri_mask, compare_op=mybir.AluOpType.is_gt,
            fill=1.0, base=0, pattern=[[-1, P]], channel_multiplier=1,
        )
        # triC[i, p] = -1000 where p > i else 0  (strictly upper).
        # Used as lhsT in a bias matmul: (triC.T @ I)[p, i] = -1000 for p > i,
        # i.e. key-row p beyond query-col i -> exp underflows to 0.
        triC = consts.tile([P, P], BF16, tag="triC", name="triC")
        nc.vector.tensor_tensor(triC, tri_mask, ident,
                                mybir.AluOpType.subtract)
        nc.vector.tensor_scalar_mul(triC, triC, -1000.0)

        xT3 = xT.rearrange("(ko p) t -> p ko t", p=P)
        w3 = {
            "q": wqT.rearrange("(ko p) (h d) -> p ko h d", p=P, h=HPC),
            "k": wkT.rearrange("(ko p) (h d) -> p ko h d", p=P, h=HPC),
            "v": wvT.rearrange("(ko p) (h d) -> p ko h d", p=P, h=HPC),
        }
        woT3 = woT.rearrange("(h p) m -> p h m", p=P)

        # resident q/k/v weights; head-0 slices first so phase A starts fast
        w_sb = {}
        for nm in ("q", "k", "v"):
            w_sb[nm] = wpool.tile([P, KD, HPC, D], BF16, tag=f"w{nm}",
                                  name=f"w{nm}")
        for h in range(HPC):
            for nm in ("q", "k", "v"):
                nc.sync.dma_start(w_sb[nm][:, :, h], w3[nm][:, :, h])

        # per-head arrays
        qT = [arrs.tile([P, BT], BF16, tag=f"qT{h}", name=f"qT{h}")
              for h in range(HPC)]
        kT = [arrs.tile([P, BT], BF16, tag=f"kT{h}", name=f"kT{h}")
              for h in range(HPC)]
        vT = [arrs.tile([P, BT], BF16, tag=f"vT{h}", name=f"vT{h}")
              for h in range(HPC)]
        v_sb = [arrs.tile([P, B * TJ, D], BF16, tag=f"v{h}", name=f"v{h}")
                for h in range(HPC)]
        outT = [arrs.tile([P, BT], BF16, tag=f"outT{h}", name=f"outT{h}")
                for h in range(HPC)]
        # transpose staging: 2 rotating [P, P] bf16 sub-buffers in PSUM
        tp = psum.tile([P, 2, P], BF16, tag="tp", name="tp")

        st = _Ctx()
        st.tpg = 0     # transpose rotation counter
        st.yc = 0      # outproj psum rotation counter

        def gen_proj(h, tch, skipchk):
            """q/k/v projection + v transposes for one 512-token chunk.
            Yields after each k-tile (3 matmuls)."""
            tsl = slice(tch * IC, (tch + 1) * IC)
            xt = xpool.tile([P, KD, IC], BF16, tag="xt", name="xt")
            for q4 in range(4):
                ks = slice(q4 * 4, (q4 + 1) * 4)
                nc.sync.dma_start(xt[:, ks], xT3[:, ks, tsl])
            ps = {nm: psum.tile([P, IC], F32, tag=f"p{i}", name=f"p{i}")
                  for i, nm in enumerate(("q", "k", "v"))}
            for kt in range(KD):
                for nm in ("q", "k", "v"):
                    nc.tensor.matmul(
                        ps[nm], w_sb[nm][:, kt, h], xt[:, kt],
                        start=(kt == 0), stop=(kt == KD - 1),
                        skip_group_check=skipchk,
                    )
                yield
            nc.scalar.copy(qT[h][:, tsl], ps["q"])
            nc.scalar.copy(kT[h][:, tsl], ps["k"])
            nc.vector.tensor_copy(vT[h][:, tsl], ps["v"])
            for i in range(IC // P):
                tt0 = tch * (IC // P) + i   # global token tile 0..31
                g = st.tpg
                st.tpg += 1
                nc.tensor.transpose(
                    tp[:, g % 2], vT[h][:, tt0 * P:(tt0 + 1) * P], ident)
                nc.vector.tensor_copy(v_sb[h][:, tt0], tp[:, g % 2])

        def gen_attn(h, b, ic, srot, ostag):
            """Attention for one 512-query chunk of head h.
            S/exp issued LA tiles ahead; yields after each PV."""
            isl = slice(b * T + ic * IC, b * T + (ic + 1) * IC)
            nj = ic * 4 + 4          # causal: j tiles 0..nj-1
            ps_o = psum.tile([P, IC], F32, tag=ostag, name=ostag)
            pt_acc = smalls.tile([P, IC], BF16, tag="ptacc")

            def s_exp(j):
                m = j - ic * 4
                lo = max(m, 0) * P   # cols < lo fully masked
                tag = srot[j % len(srot)]
                ps_s = psum.tile([P, IC], F32, tag=tag, name=tag)
                nc.tensor.matmul(
                    ps_s[:, lo:],
                    kT[h][:, b * T + j * P: b * T + (j + 1) * P],
                    qT[h][:, b * T + ic * IC + lo: b * T + (ic + 1) * IC],
                    start=True, stop=(m < 0), skip_group_check=True,
                )
                if m >= 0:
                    nc.tensor.matmul(
                        ps_s[:, lo:lo + P], triC, ident,
                        start=False, stop=True, skip_group_check=True,
                    )
                pt = ptpool.tile([P, IC], BF16, tag="pt", name="pt")
                nc.scalar.activation(
                    pt[:, lo:], ps_s[:, lo:],
                    mybir.ActivationFunctionType.Exp, scale=scale)
                return pt, lo

            pts = [s_exp(j) for j in range(min(LA, nj))]
            for j in range(nj):
                if j + LA < nj:
                    pts.append(s_exp(j + LA))
                pt, lo = pts[j]
                nc.tensor.matmul(
                    ps_o[:, lo:], v_sb[h][:, b * TJ + j], pt[:, lo:],
                    start=(j == 0), stop=(j == nj - 1),
                    skip_group_check=True,
                )
                # denominator accumulation, off the PE critical path
                if j == 0:
                    nc.vector.tensor_copy(pt_acc, pt)
                else:
                    nc.vector.tensor_tensor(
                        pt_acc[:, lo:], pt_acc[:, lo:], pt[:, lo:],
                        mybir.AluOpType.add)
                yield
            # free ps_o fast with an unnormalized copy; normalize later
            o_u = smalls.tile([P, IC], BF16, tag="ou")
            nc.scalar.copy(o_u, ps_o)
            bc = smalls.tile([P, IC], F32, tag="bc")
            nc.gpsimd.partition_all_reduce(bc, pt_acc, P, ReduceOp.add)
            rb = smalls.tile([P, IC], F32, tag="rb")
            nc.vector.reciprocal_approx_fast(out=rb, in_=bc)
            nc.vector.tensor_tensor(
                outT[h][:, isl], o_u, rb, mybir.AluOpType.mult)

        def gen_outproj(b, ic):
            """y tiles (all 2048 out-channels) for one 512-token range.
            Yields after each y tile (2 matmuls)."""
            t0 = (b * T + ic * IC) // P
            for mc in range(D_MODEL // IC):
                msl = slice(mc * IC, (mc + 1) * IC)
                for tl in range(IC // P):
                    tt = t0 + tl
                    yc = st.yc
                    st.yc += 1
                    ps_y = psum.tile([P, IC], F32, tag=f"y{yc % 2}",
                                     name=f"y{yc % 2}")
                    for h in range(HPC):
                        nc.tensor.matmul(
                            ps_y, outT[h][:, tt * P:(tt + 1) * P],
                            wo_sb[:, h, msl],
                            start=(h == 0), stop=(h == HPC - 1),
                            skip_group_check=True,
                        )
                    y_sb = ypool.tile([P, IC], F16, tag="y", name="y")
                    if yc % 2:
                        nc.scalar.copy(y_sb, ps_y)
                    else:
                        nc.vector.tensor_copy(y_sb, ps_y)
                    nc.sync.dma_start(y[tt * P:(tt + 1) * P, msl], y_sb)
                    yield

        # ---- phase A: head-0 projections ----
        for tch in range(TCH):
            _run_gen(gen_proj(0, tch, False))

        # wo needed from phase C; queue its DMA behind the x stream
        wo_sb = wpool.tile([P, HPC, D_MODEL], BF16, tag="wo", name="wo")
        nc.sync.dma_start(wo_sb, woT3)

        # ---- phase B: head-1 projections x head-0 attention ----
        chunks = [(bb, ii) for bb in range(B) for ii in range(NI)]
        for tch in range(TCH):
            bb, ii = chunks[tch]
            _drive(gen_proj(1, tch, True), KD,
                   gen_attn(0, bb, ii, ("y0", "y1", "t1"), "t0"), ii * 4 + 4)

        # ---- phase C: head-1 attention x out-projection (2 chunks back) ----
        pending = []
        for bb, ii in chunks:
            ga = gen_attn(1, bb, ii, ("p0", "p1", "p2"), "t1")
            if len(pending) >= 2:
                pb, pi = pending.pop(0)
                _drive(gen_outproj(pb, pi), 16, ga, ii * 4 + 4)
            else:
                _run_gen(ga)
            pending.append((bb, ii))
        for pb, pi in pending:
            _run_gen(gen_outproj(pb, pi))


def _build(cfg=None):
    nc = bacc.Bacc("TRN2", target_bir_lowering=False, debug=False,
                   num_devices=NCORES)
    xT = nc.dram_tensor("xT", [D_MODEL, BT], BF16, kind="ExternalInput").ap()
    wqT = nc.dram_tensor("wqT", [D_MODEL, HPC * D], BF16,
                         kind="ExternalInput").ap()
    wkT = nc.dram_tensor("wkT", [D_MODEL, HPC * D], BF16,
                         kind="ExternalInput").ap()
    wvT = nc.dram_tensor("wvT", [D_MODEL, HPC * D], BF16,
                         kind="ExternalInput").ap()
    woT = nc.dram_tensor("woT", [HPC * D, D_MODEL], BF16,
                         kind="ExternalInput").ap()
    y = nc.dram_tensor("y", [BT, D_MODEL], F16, kind="ExternalOutput").ap()
    with tile.TileContext(nc) as tc:
        _emit(tc, xT, wqT, wkT, wvT, woT, y)
    nc.compile()
    return nc


def _prep_inputs(x, Wq, Wk, Wv, Wo, cfg=None):
    bf = ml_dtypes.bfloat16
    xT = np.ascontiguousarray(
        np.asarray(x, np.float32).reshape(BT, D_MODEL).T).astype(bf)
    in_maps = []
    for c in range(NCORES):
        rows = slice(c * HPC * D, (c + 1) * HPC * D)
        in_maps.append({
            "xT": xT,
            "wqT": np.ascontiguousarray(np.asarray(Wq)[rows].T).astype(bf),
            "wkT": np.ascontiguousarray(np.asarray(Wk)[rows].T).astype(bf),
            "wvT": np.ascontiguousarray(np.asarray(Wv)[rows].T).astype(bf),
            "woT": np.ascontiguousarray(np.asarray(Wo)[:, rows].T).astype(bf),
        })
    return in_maps


def run(x, Wq, Wk, Wv, Wo, cfg=None, trace=False):
    nc = _build(cfg)
    in_maps = _prep_inputs(x, Wq, Wk, Wv, Wo, cfg)
    try:
        res = run_bass_kernel_spmd(nc, in_maps, core_ids=list(range(NCORES)),
                                   trace=trace)
    except Exception:
        res = run_bass_kernel_spmd(nc, in_maps, core_ids=list(range(NCORES)),
                                   trace=trace)
    y = np.zeros((BT, D_MODEL), np.float32)
    for r in res.results:
        y += np.asarray(r["y"], np.float32)
    return y.reshape(B, T, D_MODEL), res


def kernel(x, Wq, Wk, Wv, Wo):
    y, _ = run(x, Wq, Wk, Wv, Wo)
    return y


# revision 13
# speedup vs baseline: 1.6495x; 1.0015x over previous
"""Causal self-attention (B=2, T=2048, D=2048, 16 heads) on 8 trn2 cores.

Sharding: tensor-parallel over heads — 2 heads per core. Each core computes
q/k/v projections for its 2 heads (column-parallel), causal attention per
head, and a partial output projection (row-parallel). Host sums the 8
partial outputs.

v4: generator-based fine-grained interleave so the PE never waits on the
scalar-engine exp:
  A: head-0 q/k/v projections (+ v transposes via PE)
  B: head-1 projection matmuls interleaved per-k-tile with head-0
     attention tiles (S issued 2 tiles ahead of PV)
  C: head-1 attention interleaved per-tile with out-projection of the
     token range two chunks back (hides the gpsimd denominator chain)
Causal mask applied on the PE: diagonal S tiles accumulate a constant
strictly-lower -1000 bias via a second matmul, so exp underflows to 0 and
no vector-engine op sits between exp and the PV matmul. Softmax
denominator via gpsimd partition_all_reduce, off the critical path
(ps_o is freed by an unnormalized copy; normalize happens later on DVE).
All matmul inputs bf16, PSUM f32, output partial y in fp16.
"""

import math
from contextlib import ExitStack

import numpy as np
import ml_dtypes

import concourse.bass as bass
import concourse.mybir as mybir
import concourse.tile as tile
from concourse import bacc
from concourse.bass_isa import ReduceOp
from concourse.bass_utils import run_bass_kernel_spmd
from concourse.masks import make_identity

P = 128
D_MODEL = 2048
NUM_HEADS = 16
D = 128            # head dim
B, T = 2, 2048
BT = B * T         # 4096
NCORES = 8
HPC = NUM_HEADS // NCORES   # 2 heads per core
KD = D_MODEL // P           # 16 d_model tiles
TJ = T // P                 # 16 key tiles per batch
IC = 512                    # query chunk width
NI = T // IC                # 4 query chunks per batch
TCH = BT // IC              # 8 token chunks for projections
LA = 2                      # S-tile lookahead ahead of PV consumption

F32 = mybir.dt.float32
BF16 = mybir.dt.bfloat16
F16 = mybir.dt.float16

# kept for test.py compat; v4 is bf16-everywhere and ignores these knobs
CFG_SAFE = dict()
CFG_FAST = dict()
CFG_F32R = dict()


class _Ctx:
    pass


def _run_gen(g):
    for _ in g:
        pass


def _drive(g_part, n_part, g_att, n_att):
    """Interleave partner units with attention units (Bresenham spread),
    attention first within each step; drain both at the end."""
    done = 0
    for s in range(n_part):
        want = ((s + 1) * n_att) // n_part
        while done < want:
            next(g_att, None)
            done += 1
        next(g_part, None)
    _run_gen(g_part)
    _run_gen(g_att)


def _emit(tc, xT, wqT, wkT, wvT, woT, y):
    nc = tc.nc
    scale = 1.0 / math.sqrt(D)

    with ExitStack() as ctx:
        consts = ctx.enter_context(tc.tile_pool(name="consts", bufs=1))
        wpool = ctx.enter_context(tc.tile_pool(name="wpool", bufs=1))
        xpool = ctx.enter_context(tc.tile_pool(name="xpool", bufs=2))
        arrs = ctx.enter_context(tc.tile_pool(name="arrs", bufs=1))
        ptpool = ctx.enter_context(tc.tile_pool(name="ptpool", bufs=6))
        smalls = ctx.enter_context(tc.tile_pool(name="smalls", bufs=4))
        ypool = ctx.enter_context(tc.tile_pool(name="ypool", bufs=4))
        psum = ctx.enter_context(tc.tile_pool(name="psum", bufs=1, space="PSUM"))

        ident = consts.tile([P, P], BF16, tag="ident", name="ident")
        make_identity(nc, ident)

        # tri_mask[p, i] = 1.0 if i >= p else 0 (upper triangular keep)
        tri_mask = consts.tile([P, P], BF16, tag="trimask", name="trimask")
        nc.gpsimd.memset(tri_mask, 0.0)
        nc.gpsimd.affine_select(
            out=tri_mask, in_=tri_mask, compare_op=mybir.AluOpType.is_gt,
            fill=1.0, base=0, pattern=[[-1, P]], channel_multiplier=1,
        )
        # triC[i, p] = -1000 where p > i else 0  (strictly upper).
        # Used as lhsT in a bias matmul: (triC.T @ I)[p, i] = -1000 for p > i,
        # i.e. key-row p beyond query-col i -> exp underflows to 0.
        triC = consts.tile([P, P], BF16, tag="triC", name="triC")
        nc.vector.tensor_tensor(triC, tri_mask, ident,
                                mybir.AluOpType.subtract)
        nc.vector.tensor_scalar_mul(triC, triC, -1000.0)

        xT3 = xT.rearrange("(ko p) t -> p ko t", p=P)
        w3 = {
            "q": wqT.rearrange("(ko p) (h d) -> p ko h d", p=P, h=HPC),
            "k": wkT.rearrange("(ko p) (h d) -> p ko h d", p=P, h=HPC),
            "v": wvT.rearrange("(ko p) (h d) -> p ko h d", p=P, h=HPC),
        }
        woT3 = woT.rearrange("(h p) m -> p h m", p=P)

        # resident q/k/v weights; head-0 k-tile halves first so phase A's
        # first matmuls aren't queued behind the full weight load
        w_sb = {}
        for nm in ("q", "k", "v"):
            w_sb[nm] = wpool.tile([P, KD, HPC, D], BF16, tag=f"w{nm}",
                                  name=f"w{nm}")
        for ks in (slice(0, KD // 2), slice(KD // 2, KD)):
            for nm in ("q", "k", "v"):
                nc.sync.dma_start(w_sb[nm][:, ks, 0], w3[nm][:, ks, 0])

        # per-head arrays
        qT = [arrs.tile([P, BT], BF16, tag=f"qT{h}", name=f"qT{h}")
              for h in range(HPC)]
        kT = [arrs.tile([P, BT], BF16, tag=f"kT{h}", name=f"kT{h}")
              for h in range(HPC)]
        vT = [arrs.tile([P, BT], BF16, tag=f"vT{h}", name=f"vT{h}")
              for h in range(HPC)]
        v_sb = [arrs.tile([P, B * TJ, D], BF16, tag=f"v{h}", name=f"v{h}")
                for h in range(HPC)]
        outT = [arrs.tile([P, BT], BF16, tag=f"outT{h}", name=f"outT{h}")
                for h in range(HPC)]
        # transpose staging: 2 rotating [P, P] bf16 sub-buffers in PSUM
        tp = psum.tile([P, 2, P], BF16, tag="tp", name="tp")

        st = _Ctx()
        st.tpg = 0     # transpose rotation counter
        st.yc = 0      # outproj psum rotation counter

        def gen_proj(h, tch, skipchk):
            """q/k/v projection + v transposes for one 512-token chunk.
            Yields after each k-tile (3 matmuls)."""
            tsl = slice(tch * IC, (tch + 1) * IC)
            xt = xpool.tile([P, KD, IC], BF16, tag="xt", name="xt")
            for q4 in range(4):
                ks = slice(q4 * 4, (q4 + 1) * 4)
                nc.sync.dma_start(xt[:, ks], xT3[:, ks, tsl])
            ps = {nm: psum.tile([P, IC], F32, tag=f"p{i}", name=f"p{i}")
                  for i, nm in enumerate(("q", "k", "v"))}
            for kt in range(KD):
                for nm in ("q", "k", "v"):
                    nc.tensor.matmul(
                        ps[nm], w_sb[nm][:, kt, h], xt[:, kt],
                        start=(kt == 0), stop=(kt == KD - 1),
                        skip_group_check=skipchk,
                    )
                yield
            nc.scalar.copy(qT[h][:, tsl], ps["q"])
            nc.scalar.copy(kT[h][:, tsl], ps["k"])
            nc.vector.tensor_copy(vT[h][:, tsl], ps["v"])
            for i in range(IC // P):
                tt0 = tch * (IC // P) + i   # global token tile 0..31
                g = st.tpg
                st.tpg += 1
                nc.tensor.transpose(
                    tp[:, g % 2], vT[h][:, tt0 * P:(tt0 + 1) * P], ident)
                nc.vector.tensor_copy(v_sb[h][:, tt0], tp[:, g % 2])

        def gen_attn(h, b, ic, srot, ostag):
            """Attention for one 512-query chunk of head h.
            S/exp issued LA tiles ahead; yields after each PV."""
            isl = slice(b * T + ic * IC, b * T + (ic + 1) * IC)
            nj = ic * 4 + 4          # causal: j tiles 0..nj-1
            ps_o = psum.tile([P, IC], F32, tag=ostag, name=ostag)
            pt_acc = smalls.tile([P, IC], BF16, tag="ptacc")

            def s_exp(j):
                m = j - ic * 4
                lo = max(m, 0) * P   # cols < lo fully masked
                tag = srot[j % len(srot)]
                ps_s = psum.tile([P, IC], F32, tag=tag, name=tag)
                nc.tensor.matmul(
                    ps_s[:, lo:],
                    kT[h][:, b * T + j * P: b * T + (j + 1) * P],
                    qT[h][:, b * T + ic * IC + lo: b * T + (ic + 1) * IC],
                    start=True, stop=(m < 0), skip_group_check=True,
                )
                if m >= 0:
                    nc.tensor.matmul(
                        ps_s[:, lo:lo + P], triC, ident,
                        start=False, stop=True, skip_group_check=True,
                    )
                pt = ptpool.tile([P, IC], BF16, tag="pt", name="pt")
                nc.scalar.activation(
                    pt[:, lo:], ps_s[:, lo:],
                    mybir.ActivationFunctionType.Exp, scale=scale)
                return pt, lo

            pts = [s_exp(j) for j in range(min(LA, nj))]
            for j in range(nj):
                if j + LA < nj:
                    pts.append(s_exp(j + LA))
                pt, lo = pts[j]
                nc.tensor.matmul(
                    ps_o[:, lo:], v_sb[h][:, b * TJ + j], pt[:, lo:],
                    start=(j == 0), stop=(j == nj - 1),
                    skip_group_check=True,
                )
                # denominator accumulation, off the PE critical path
                if j == 0:
                    nc.vector.tensor_copy(pt_acc, pt)
                else:
                    nc.vector.tensor_tensor(
                        pt_acc[:, lo:], pt_acc[:, lo:], pt[:, lo:],
                        mybir.AluOpType.add)
                yield
            # free ps_o fast with an unnormalized copy; normalize later
            o_u = smalls.tile([P, IC], BF16, tag="ou")
            nc.scalar.copy(o_u, ps_o)
            bc = smalls.tile([P, IC], F32, tag="bc")
            nc.gpsimd.partition_all_reduce(bc, pt_acc, P, ReduceOp.add)
            rb = smalls.tile([P, IC], F32, tag="rb")
            nc.vector.reciprocal_approx_fast(out=rb, in_=bc)
            nc.vector.tensor_tensor(
                outT[h][:, isl], o_u, rb, mybir.AluOpType.mult)

        def gen_outproj(b, ic):
            """y tiles (all 2048 out-channels) for one 512-token range.
            Yields after each y tile (2 matmuls)."""
            t0 = (b * T + ic * IC) // P
            for mc in range(D_MODEL // IC):
                msl = slice(mc * IC, (mc + 1) * IC)
                for tl in range(IC // P):
                    tt = t0 + tl
                    yc = st.yc
                    st.yc += 1
                    ps_y = psum.tile([P, IC], F32, tag=f"y{yc % 2}",
                                     name=f"y{yc % 2}")
                    for h in range(HPC):
                        nc.tensor.matmul(
                            ps_y, outT[h][:, tt * P:(tt + 1) * P],
                            wo_sb[:, h, msl],
                            start=(h == 0), stop=(h == HPC - 1),
                            skip_group_check=True,
                        )
                    y_sb = ypool.tile([P, IC], F16, tag="y", name="y")
                    if yc % 2:
                        nc.scalar.copy(y_sb, ps_y)
                    else:
                        nc.vector.tensor_copy(y_sb, ps_y)
                    nc.sync.dma_start(y[tt * P:(tt + 1) * P, msl], y_sb)
                    yield

        # ---- phase A: head-0 projections ----
        for tch in range(TCH):
            _run_gen(gen_proj(0, tch, False))
            if tch == 0:
                # head-1 weights: queue behind tch0's x stream so they
                # don't delay the first matmuls but arrive before phase B
                for nm in ("q", "k", "v"):
                    nc.sync.dma_start(w_sb[nm][:, :, 1], w3[nm][:, :, 1])

        # wo needed from phase C; queue its DMA behind the x stream
        wo_sb = wpool.tile([P, HPC, D_MODEL], BF16, tag="wo", name="wo")
        nc.sync.dma_start(wo_sb, woT3)

        # ---- phase B: head-1 projections x head-0 attention ----
        chunks = [(bb, ii) for bb in range(B) for ii in range(NI)]
        for tch in range(TCH):
            bb, ii = chunks[tch]
            _drive(gen_proj(1, tch, True), KD,
                   gen_attn(0, bb, ii, ("y0", "y1", "t1"), "t0"), ii * 4 + 4)

        # ---- phase C: head-1 attention x out-projection (2 chunks back) ----
        pending = []
        for bb, ii in chunks:
            ga = gen_attn(1, bb, ii, ("p0", "p1", "p2"), "t1")
            if len(pending) >= 2:
                pb, pi = pending.pop(0)
                _drive(gen_outproj(pb, pi), 16, ga, ii * 4 + 4)
            else:
                _run_gen(ga)
            pending.append((bb, ii))
        for pb, pi in pending:
            _run_gen(gen_outproj(pb, pi))


def _build(cfg=None):
    nc = bacc.Bacc("TRN2", target_bir_lowering=False, debug=False,
                   num_devices=NCORES)
    xT = nc.dram_tensor("xT", [D_MODEL, BT], BF16, kind="ExternalInput").ap()
    wqT = nc.dram_tensor("wqT", [D_MODEL, HPC * D], BF16,
                         kind="ExternalInput").ap()
    wkT = nc.dram_tensor("wkT", [D_MODEL, HPC * D], BF16,
                         kind="ExternalInput").ap()
    wvT = nc.dram_tensor("wvT", [D_MODEL, HPC * D], BF16,
                         kind="ExternalInput").ap()
    woT = nc.dram_tensor("woT", [HPC * D, D_MODEL], BF16,
                         kind="ExternalInput").ap()
    y = nc.dram_tensor("y", [BT, D_MODEL], F16, kind="ExternalOutput").ap()
    with tile.TileContext(nc) as tc:
        _emit(tc, xT, wqT, wkT, wvT, woT, y)
    nc.compile()
    return nc


def _prep_inputs(x, Wq, Wk, Wv, Wo, cfg=None):
    bf = ml_dtypes.bfloat16
    xT = np.ascontiguousarray(
        np.asarray(x, np.float32).reshape(BT, D_MODEL).T).astype(bf)
    in_maps = []
    for c in range(NCORES):
        rows = slice(c * HPC * D, (c + 1) * HPC * D)
        in_maps.append({
            "xT": xT,
            "wqT": np.ascontiguousarray(np.asarray(Wq)[rows].T).astype(bf),
            "wkT": np.ascontiguousarray(np.asarray(Wk)[rows].T).astype(bf),
            "wvT": np.ascontiguousarray(np.asarray(Wv)[rows].T).astype(bf),
            "woT": np.ascontiguousarray(np.asarray(Wo)[:, rows].T).astype(bf),
        })
    return in_maps


def run(x, Wq, Wk, Wv, Wo, cfg=None, trace=False):
    nc = _build(cfg)
    in_maps = _prep_inputs(x, Wq, Wk, Wv, Wo, cfg)
    try:
        res = run_bass_kernel_spmd(nc, in_maps, core_ids=list(range(NCORES)),
                                   trace=trace)
    except Exception:
        res = run_bass_kernel_spmd(nc, in_maps, core_ids=list(range(NCORES)),
                                   trace=trace)
    y = np.zeros((BT, D_MODEL), np.float32)
    for r in res.results:
        y += np.asarray(r["y"], np.float32)
    return y.reshape(B, T, D_MODEL), res


def kernel(x, Wq, Wk, Wv, Wo):
    y, _ = run(x, Wq, Wk, Wv, Wo)
    return y


# revision 14
# speedup vs baseline: 1.7238x; 1.0451x over previous
"""Causal self-attention (B=2, T=2048, D=2048, 16 heads) on 8 trn2 cores.

Sharding: tensor-parallel over heads — 2 heads per core. Each core computes
q/k/v projections for its 2 heads (column-parallel), causal attention per
head, and a partial output projection (row-parallel). Host sums the 8
partial outputs.

v5: generator-based fine-grained interleave so the PE never waits on the
scalar-engine exp:
  A: head-0 q/k/v projections (+ v transposes via PE)
  B: head-1 projection matmuls interleaved per-k-tile with head-0
     attention tiles (S issued 2 tiles ahead of PV)
  C: head-1 attention interleaved per-tile with out-projection of the
     token range two chunks back
Causal mask applied on the PE: diagonal S tiles accumulate a constant
strictly-lower -1000 bias via a second matmul, so exp underflows to 0 and
nothing sits between exp and the PV matmul. Softmax denominator via
gpsimd partition_all_reduce; its reciprocal+multiply are emitted one
chunk later (closure queue) so they never head-of-line-block the DVE
FIFO while gpsimd runs. Host pre-arranges x and weights so every DMA is
contiguous per partition. All matmul inputs bf16, PSUM f32, partial y
in fp16.
"""

import math
from contextlib import ExitStack

import numpy as np
import ml_dtypes

import concourse.bass as bass
import concourse.mybir as mybir
import concourse.tile as tile
from concourse import bacc
from concourse.bass_isa import ReduceOp
from concourse.bass_utils import run_bass_kernel_spmd
from concourse.masks import make_identity

P = 128
D_MODEL = 2048
NUM_HEADS = 16
D = 128            # head dim
B, T = 2, 2048
BT = B * T         # 4096
NCORES = 8
HPC = NUM_HEADS // NCORES   # 2 heads per core
KD = D_MODEL // P           # 16 d_model tiles
TJ = T // P                 # 16 key tiles per batch
IC = 512                    # query chunk width
NI = T // IC                # 4 query chunks per batch
TCH = BT // IC              # 8 token chunks for projections
LA = 2                      # S-tile lookahead ahead of PV consumption

F32 = mybir.dt.float32
BF16 = mybir.dt.bfloat16
F16 = mybir.dt.float16

# kept for test.py compat; v5 is bf16-everywhere and ignores these knobs
CFG_SAFE = dict()
CFG_FAST = dict()
CFG_F32R = dict()


class _Ctx:
    pass


def _run_gen(g):
    for _ in g:
        pass


def _drive(g_part, n_part, g_att, n_att):
    """Interleave partner units with attention units (Bresenham spread),
    attention first within each step; drain both at the end."""
    done = 0
    for s in range(n_part):
        want = ((s + 1) * n_att) // n_part
        while done < want:
            next(g_att, None)
            done += 1
        next(g_part, None)
    _run_gen(g_part)
    _run_gen(g_att)


def _emit(tc, xT, wqT, wkT, wvT, woT, y):
    nc = tc.nc
    scale = 1.0 / math.sqrt(D)

    with ExitStack() as ctx:
        consts = ctx.enter_context(tc.tile_pool(name="consts", bufs=1))
        wpool = ctx.enter_context(tc.tile_pool(name="wpool", bufs=1))
        xpool = ctx.enter_context(tc.tile_pool(name="xpool", bufs=2))
        arrs = ctx.enter_context(tc.tile_pool(name="arrs", bufs=1))
        ptpool = ctx.enter_context(tc.tile_pool(name="ptpool", bufs=6))
        smalls = ctx.enter_context(tc.tile_pool(name="smalls", bufs=4))
        ypool = ctx.enter_context(tc.tile_pool(name="ypool", bufs=6))
        psum = ctx.enter_context(tc.tile_pool(name="psum", bufs=1, space="PSUM"))

        ident = consts.tile([P, P], BF16, tag="ident", name="ident")
        make_identity(nc, ident)

        # tri_mask[p, i] = 1.0 if i >= p else 0 (upper triangular keep)
        tri_mask = consts.tile([P, P], BF16, tag="trimask", name="trimask")
        nc.gpsimd.memset(tri_mask, 0.0)
        nc.gpsimd.affine_select(
            out=tri_mask, in_=tri_mask, compare_op=mybir.AluOpType.is_gt,
            fill=1.0, base=0, pattern=[[-1, P]], channel_multiplier=1,
        )
        # triC[i, p] = -1000 where p > i else 0  (strictly upper).
        # Used as lhsT in a bias matmul: (triC.T @ I)[p, i] = -1000 for p > i,
        # i.e. key-row p beyond query-col i -> exp underflows to 0.
        triC = consts.tile([P, P], BF16, tag="triC", name="triC")
        nc.vector.tensor_tensor(triC, tri_mask, ident,
                                mybir.AluOpType.subtract)
        nc.vector.tensor_scalar_mul(triC, triC, -1000.0)

        # host-prepped layouts (contiguous per-partition DMA lines):
        # xT  [TCH*P, KD*IC]  x4[tch, p, kt, i]
        # w*T [HPC*P, KD*D]   w4[h, p, kt, d]
        # woT [HPC*P, D_MODEL]
        x4 = xT.rearrange("(c p) (k i) -> c p k i", p=P, k=KD)
        w4 = {
            "q": wqT.rearrange("(h p) (k d) -> h p k d", p=P, k=KD),
            "k": wkT.rearrange("(h p) (k d) -> h p k d", p=P, k=KD),
            "v": wvT.rearrange("(h p) (k d) -> h p k d", p=P, k=KD),
        }
        woT3 = woT.rearrange("(h p) m -> p h m", p=P)

        # resident q/k/v weights; head-0 first so phase A starts fast
        w_sb = {}
        for nm in ("q", "k", "v"):
            w_sb[nm] = wpool.tile([P, KD, HPC, D], BF16, tag=f"w{nm}",
                                  name=f"w{nm}")
        for nm in ("q", "k", "v"):
            nc.sync.dma_start(w_sb[nm][:, :, 0], w4[nm][0])

        # per-head arrays
        qT = [arrs.tile([P, BT], BF16, tag=f"qT{h}", name=f"qT{h}")
              for h in range(HPC)]
        kT = [arrs.tile([P, BT], BF16, tag=f"kT{h}", name=f"kT{h}")
              for h in range(HPC)]
        vT = [arrs.tile([P, BT], BF16, tag=f"vT{h}", name=f"vT{h}")
              for h in range(HPC)]
        v_sb = [arrs.tile([P, B * TJ, D], BF16, tag=f"v{h}", name=f"v{h}")
                for h in range(HPC)]
        outT = [arrs.tile([P, BT], BF16, tag=f"outT{h}", name=f"outT{h}")
                for h in range(HPC)]
        # transpose staging: 2 rotating [P, P] bf16 sub-buffers in PSUM
        tp = psum.tile([P, 2, P], BF16, tag="tp", name="tp")

        st = _Ctx()
        st.tpg = 0          # transpose rotation counter
        st.yc = 0           # outproj psum rotation counter
        st.norm_q = []      # deferred normalize closures

        def flush_norms(keep):
            while len(st.norm_q) > keep:
                st.norm_q.pop(0)()

        def gen_proj(h, tch, skipchk):
            """q/k/v projection + v transposes for one 512-token chunk.
            Yields after each k-tile (3 matmuls)."""
            tsl = slice(tch * IC, (tch + 1) * IC)
            xt = xpool.tile([P, KD, IC], BF16, tag="xt", name="xt")
            for q4 in range(4):
                ks = slice(q4 * 4, (q4 + 1) * 4)
                nc.sync.dma_start(xt[:, ks], x4[tch, :, ks])
            ps = {nm: psum.tile([P, IC], F32, tag=f"p{i}", name=f"p{i}")
                  for i, nm in enumerate(("q", "k", "v"))}
            for kt in range(KD):
                for nm in ("q", "k", "v"):
                    nc.tensor.matmul(
                        ps[nm], w_sb[nm][:, kt, h], xt[:, kt],
                        start=(kt == 0), stop=(kt == KD - 1),
                        skip_group_check=skipchk,
                    )
                yield
            nc.vector.tensor_copy(qT[h][:, tsl], ps["q"])
            nc.vector.tensor_copy(kT[h][:, tsl], ps["k"])
            nc.vector.tensor_copy(vT[h][:, tsl], ps["v"])
            for i in range(IC // P):
                tt0 = tch * (IC // P) + i   # global token tile 0..31
                g = st.tpg
                st.tpg += 1
                nc.tensor.transpose(
                    tp[:, g % 2], vT[h][:, tt0 * P:(tt0 + 1) * P], ident)
                nc.vector.tensor_copy(v_sb[h][:, tt0], tp[:, g % 2])

        def gen_attn(h, b, ic, srot, ostag):
            """Attention for one 512-query chunk of head h.
            S/exp issued LA tiles ahead; yields after each PV.
            Normalize (recip+mult) is deferred via st.norm_q."""
            isl = slice(b * T + ic * IC, b * T + (ic + 1) * IC)
            nj = ic * 4 + 4          # causal: j tiles 0..nj-1
            ps_o = psum.tile([P, IC], F32, tag=ostag, name=ostag)
            pt_acc = smalls.tile([P, IC], BF16, tag="ptacc")

            def s_exp(j):
                m = j - ic * 4
                lo = max(m, 0) * P   # cols < lo fully masked
                tag = srot[j % len(srot)]
                ps_s = psum.tile([P, IC], F32, tag=tag, name=tag)
                nc.tensor.matmul(
                    ps_s[:, lo:],
                    kT[h][:, b * T + j * P: b * T + (j + 1) * P],
                    qT[h][:, b * T + ic * IC + lo: b * T + (ic + 1) * IC],
                    start=True, stop=(m < 0), skip_group_check=True,
                )
                if m >= 0:
                    nc.tensor.matmul(
                        ps_s[:, lo:lo + P], triC, ident,
                        start=False, stop=True, skip_group_check=True,
                    )
                pt = ptpool.tile([P, IC], BF16, tag="pt", name="pt")
                nc.scalar.activation(
                    pt[:, lo:], ps_s[:, lo:],
                    mybir.ActivationFunctionType.Exp, scale=scale)
                return pt, lo

            pts = [s_exp(j) for j in range(min(LA, nj))]
            for j in range(nj):
                if j + LA < nj:
                    pts.append(s_exp(j + LA))
                pt, lo = pts[j]
                nc.tensor.matmul(
                    ps_o[:, lo:], v_sb[h][:, b * TJ + j], pt[:, lo:],
                    start=(j == 0), stop=(j == nj - 1),
                    skip_group_check=True,
                )
                # denominator accumulation, off the PE critical path
                if j == 0:
                    nc.vector.tensor_copy(pt_acc, pt)
                else:
                    nc.vector.tensor_tensor(
                        pt_acc[:, lo:], pt_acc[:, lo:], pt[:, lo:],
                        mybir.AluOpType.add)
                yield
            # free ps_o fast with an unnormalized copy; launch the gpsimd
            # all-reduce now, but defer recip+mult (DVE) so they don't
            # head-of-line-block the DVE queue behind gpsimd
            o_u = smalls.tile([P, IC], BF16, tag="ou")
            nc.vector.tensor_copy(o_u, ps_o)
            bc = smalls.tile([P, IC], F32, tag="bc")
            nc.gpsimd.partition_all_reduce(bc, pt_acc, P, ReduceOp.add)

            def norm():
                rb = smalls.tile([P, IC], F32, tag="rb")
                nc.vector.reciprocal_approx_fast(out=rb, in_=bc)
                nc.vector.tensor_tensor(
                    outT[h][:, isl], o_u, rb, mybir.AluOpType.mult)

            st.norm_q.append(norm)

        def gen_outproj(b, ic):
            """y tiles (all 2048 out-channels) for one 512-token range.
            Yields after each y tile (2 matmuls)."""
            t0 = (b * T + ic * IC) // P
            for mc in range(D_MODEL // IC):
                msl = slice(mc * IC, (mc + 1) * IC)
                for tl in range(IC // P):
                    tt = t0 + tl
                    yc = st.yc
                    st.yc += 1
                    ps_y = psum.tile([P, IC], F32, tag=f"y{yc % 2}",
                                     name=f"y{yc % 2}")
                    for h in range(HPC):
                        nc.tensor.matmul(
                            ps_y, outT[h][:, tt * P:(tt + 1) * P],
                            wo_sb[:, h, msl],
                            start=(h == 0), stop=(h == HPC - 1),
                            skip_group_check=True,
                        )
                    y_sb = ypool.tile([P, IC], F16, tag="y", name="y")
                    if yc % 3 == 2:
                        nc.scalar.copy(y_sb, ps_y)
                    else:
                        nc.vector.tensor_copy(y_sb, ps_y)
                    nc.sync.dma_start(y[tt * P:(tt + 1) * P, msl], y_sb)
                    yield

        # ---- phase A: head-0 projections ----
        for tch in range(TCH):
            _run_gen(gen_proj(0, tch, False))
            if tch == 0:
                # head-1 weights: behind tch0's x stream, before phase B
                for nm in ("q", "k", "v"):
                    nc.sync.dma_start(w_sb[nm][:, :, 1], w4[nm][1])

        # wo needed from phase C; queue its DMA behind the x stream
        wo_sb = wpool.tile([P, HPC, D_MODEL], BF16, tag="wo", name="wo")
        nc.sync.dma_start(wo_sb, woT3)

        # ---- phase B: head-1 projections x head-0 attention ----
        chunks = [(bb, ii) for bb in range(B) for ii in range(NI)]
        for tch in range(TCH):
            bb, ii = chunks[tch]
            _drive(gen_proj(1, tch, True), KD,
                   gen_attn(0, bb, ii, ("y0", "y1", "t1"), "t0"), ii * 4 + 4)
            flush_norms(keep=1)

        # ---- phase C: head-1 attention x out-projection (2 chunks back) ----
        pending = []
        for bb, ii in chunks:
            ga = gen_attn(1, bb, ii, ("p0", "p1", "p2"), "t1")
            if len(pending) >= 2:
                pb, pi = pending.pop(0)
                _drive(gen_outproj(pb, pi), 16, ga, ii * 4 + 4)
            else:
                _run_gen(ga)
            pending.append((bb, ii))
            flush_norms(keep=1)
        flush_norms(keep=0)
        for pb, pi in pending:
            _run_gen(gen_outproj(pb, pi))


def _build(cfg=None):
    nc = bacc.Bacc("TRN2", target_bir_lowering=False, debug=False,
                   num_devices=NCORES)
    xT = nc.dram_tensor("xT", [TCH * P, KD * IC], BF16,
                        kind="ExternalInput").ap()
    wqT = nc.dram_tensor("wqT", [HPC * P, KD * D], BF16,
                         kind="ExternalInput").ap()
    wkT = nc.dram_tensor("wkT", [HPC * P, KD * D], BF16,
                         kind="ExternalInput").ap()
    wvT = nc.dram_tensor("wvT", [HPC * P, KD * D], BF16,
                         kind="ExternalInput").ap()
    woT = nc.dram_tensor("woT", [HPC * D, D_MODEL], BF16,
                         kind="ExternalInput").ap()
    y = nc.dram_tensor("y", [BT, D_MODEL], F16, kind="ExternalOutput").ap()
    with tile.TileContext(nc) as tc:
        _emit(tc, xT, wqT, wkT, wvT, woT, y)
    nc.compile()
    return nc


def _prep_inputs(x, Wq, Wk, Wv, Wo, cfg=None):
    bf = ml_dtypes.bfloat16
    xT = np.ascontiguousarray(
        np.asarray(x, np.float32).reshape(BT, D_MODEL).T).astype(bf)
    # x4[tch, p, kt, i] = xT[kt*P + p, tch*IC + i], flattened 2D
    x4 = np.ascontiguousarray(
        xT.reshape(KD, P, TCH, IC).transpose(2, 1, 0, 3)
    ).reshape(TCH * P, KD * IC)
    in_maps = []
    for c in range(NCORES):
        rows = slice(c * HPC * D, (c + 1) * HPC * D)

        def wprep(W):
            # w4[h, p, kt, d] = W[rows][h*D + d, kt*P + p], flattened 2D
            wT = np.ascontiguousarray(np.asarray(W)[rows].T).astype(bf)
            return np.ascontiguousarray(
                wT.reshape(KD, P, HPC, D).transpose(2, 1, 0, 3)
            ).reshape(HPC * P, KD * D)

        in_maps.append({
            "xT": x4,
            "wqT": wprep(Wq),
            "wkT": wprep(Wk),
            "wvT": wprep(Wv),
            "woT": np.ascontiguousarray(
                np.asarray(Wo)[:, rows].T).astype(bf),
        })
    return in_maps


def run(x, Wq, Wk, Wv, Wo, cfg=None, trace=False):
    nc = _build(cfg)
    in_maps = _prep_inputs(x, Wq, Wk, Wv, Wo, cfg)
    try:
        res = run_bass_kernel_spmd(nc, in_maps, core_ids=list(range(NCORES)),
                                   trace=trace)
    except Exception:
        res = run_bass_kernel_spmd(nc, in_maps, core_ids=list(range(NCORES)),
                                   trace=trace)
    y = np.zeros((BT, D_MODEL), np.float32)
    for r in res.results:
        y += np.asarray(r["y"], np.float32)
    return y.reshape(B, T, D_MODEL), res


def kernel(x, Wq, Wk, Wv, Wo):
    y, _ = run(x, Wq, Wk, Wv, Wo)
    return y


# revision 20
# speedup vs baseline: 1.8919x; 1.0975x over previous
"""Causal self-attention (B=2, T=2048, D=2048, 16 heads) on 8 trn2 cores.

Sharding: tensor-parallel over heads — 2 heads per core. Each core computes
q/k/v projections for its 2 heads (column-parallel), causal attention per
head, and a partial output projection (row-parallel). Host sums the 8
partial outputs.

v5: generator-based fine-grained interleave so the PE never waits on the
scalar-engine exp:
  A: head-0 q/k/v projections (+ v transposes via PE)
  B: head-1 projection matmuls interleaved per-k-tile with head-0
     attention tiles (S issued 2 tiles ahead of PV)
  C: head-1 attention interleaved per-tile with out-projection of the
     token range two chunks back
Causal mask applied on the PE: diagonal S tiles accumulate a constant
strictly-lower -1000 bias via a second matmul, so exp underflows to 0 and
nothing sits between exp and the PV matmul. Softmax denominator via
gpsimd partition_all_reduce; its reciprocal+multiply are emitted one
chunk later (closure queue) so they never head-of-line-block the DVE
FIFO while gpsimd runs. Host pre-arranges x and weights so every DMA is
contiguous per partition. All matmul inputs bf16, PSUM f32, partial y
in fp16.
"""

import math
from contextlib import ExitStack

import numpy as np
import ml_dtypes

import concourse.bass as bass
import concourse.mybir as mybir
import concourse.tile as tile
from concourse import bacc
from concourse.bass_isa import ReduceOp
from concourse.bass_utils import run_bass_kernel_spmd
from concourse.masks import make_identity

P = 128
D_MODEL = 2048
NUM_HEADS = 16
D = 128            # head dim
B, T = 2, 2048
BT = B * T         # 4096
NCORES = 8
HPC = NUM_HEADS // NCORES   # 2 heads per core
KD = D_MODEL // P           # 16 d_model tiles
TJ = T // P                 # 16 key tiles per batch
IC = 512                    # query chunk width
NI = T // IC                # 4 query chunks per batch
TCH = BT // IC              # 8 token chunks for projections
LA = 2                      # S-tile lookahead ahead of PV consumption

F32 = mybir.dt.float32
BF16 = mybir.dt.bfloat16
F16 = mybir.dt.float16

# kept for test.py compat; v5 is bf16-everywhere and ignores these knobs
CFG_SAFE = dict()
CFG_FAST = dict()
CFG_F32R = dict()


class _Ctx:
    pass


def _run_gen(g):
    for _ in g:
        pass


def _drive(g_part, n_part, g_att, n_att):
    """Interleave partner units with attention units (Bresenham spread),
    attention first within each step; drain both at the end."""
    done = 0
    for s in range(n_part):
        want = ((s + 1) * n_att) // n_part
        while done < want:
            next(g_att, None)
            done += 1
        next(g_part, None)
    _run_gen(g_part)
    _run_gen(g_att)


def _emit(tc, xT, wqT, wkT, wvT, woT, y):
    nc = tc.nc
    scale = 1.0 / math.sqrt(D)

    with ExitStack() as ctx:
        consts = ctx.enter_context(tc.tile_pool(name="consts", bufs=1))
        wpool = ctx.enter_context(tc.tile_pool(name="wpool", bufs=1))
        xpool = ctx.enter_context(tc.tile_pool(name="xpool", bufs=3))
        arrs = ctx.enter_context(tc.tile_pool(name="arrs", bufs=1))
        ptpool = ctx.enter_context(tc.tile_pool(name="ptpool", bufs=6))
        smalls = ctx.enter_context(tc.tile_pool(name="smalls", bufs=4))
        ypool = ctx.enter_context(tc.tile_pool(name="ypool", bufs=6))
        psum = ctx.enter_context(tc.tile_pool(name="psum", bufs=1, space="PSUM"))

        ident = consts.tile([P, P], BF16, tag="ident", name="ident")
        make_identity(nc, ident)
        ones_col = consts.tile([P, 1], BF16, tag="ones", name="ones")
        nc.vector.memset(ones_col, 1.0)

        # tri_mask[p, i] = 1.0 if i >= p else 0 (upper triangular keep)
        tri_mask = consts.tile([P, P], BF16, tag="trimask", name="trimask")
        nc.gpsimd.memset(tri_mask, 0.0)
        nc.gpsimd.affine_select(
            out=tri_mask, in_=tri_mask, compare_op=mybir.AluOpType.is_gt,
            fill=1.0, base=0, pattern=[[-1, P]], channel_multiplier=1,
        )
        # triC[i, p] = -1000 where p > i else 0  (strictly upper).
        # Used as lhsT in a bias matmul: (triC.T @ I)[p, i] = -1000 for p > i,
        # i.e. key-row p beyond query-col i -> exp underflows to 0.
        triC = consts.tile([P, P], BF16, tag="triC", name="triC")
        nc.vector.tensor_tensor(triC, tri_mask, ident,
                                mybir.AluOpType.subtract)
        nc.vector.tensor_scalar_mul(triC, triC, -1000.0)

        # host-prepped layouts (contiguous per-partition DMA lines):
        # xT  [TCH*P, KD*IC]  x4[tch, p, kt, i]
        # w*T [HPC*P, KD*D]   w4[h, p, kt, d]
        # woT [HPC*P, D_MODEL]
        x4 = xT.rearrange("(c p) (k i) -> c p k i", p=P, k=KD)
        w4 = {
            "q": wqT.rearrange("(h p) (k d) -> h p k d", p=P, k=KD),
            "k": wkT.rearrange("(h p) (k d) -> h p k d", p=P, k=KD),
            "v": wvT.rearrange("(h p) (k d) -> h p k d", p=P, k=KD),
        }
        woT3 = woT.rearrange("(h p) m -> p h m", p=P)

        # resident q/k/v weights; head-0 first so phase A starts fast
        w_sb = {}
        for nm in ("q", "k", "v"):
            w_sb[nm] = wpool.tile([P, KD, HPC, D], BF16, tag=f"w{nm}",
                                  name=f"w{nm}")
        for nm in ("q", "k", "v"):
            nc.sync.dma_start(w_sb[nm][:, :, 0], w4[nm][0])

        # per-head arrays
        qT = [arrs.tile([P, BT], BF16, tag=f"qT{h}", name=f"qT{h}")
              for h in range(HPC)]
        kT = [arrs.tile([P, BT], BF16, tag=f"kT{h}", name=f"kT{h}")
              for h in range(HPC)]
        vT = [arrs.tile([P, BT], BF16, tag=f"vT{h}", name=f"vT{h}")
              for h in range(HPC)]
        v_sb = [arrs.tile([P, B * TJ, D], BF16, tag=f"v{h}", name=f"v{h}")
                for h in range(HPC)]
        outT = [arrs.tile([P, BT], BF16, tag=f"outT{h}", name=f"outT{h}")
                for h in range(HPC)]
        # transpose staging: 2 rotating [P, P] bf16 sub-buffers in PSUM
        tp = psum.tile([P, 2, P], BF16, tag="tp", name="tp")

        st = _Ctx()
        st.tpg = 0          # transpose rotation counter
        st.yc = 0           # outproj psum rotation counter
        st.norm_q = []      # deferred normalize closures

        def flush_norms(keep):
            while len(st.norm_q) > keep:
                st.norm_q.pop(0)()

        def new_xt(tch):
            xt = xpool.tile([P, KD, IC], BF16, tag="xt", name="xt")
            for q4 in range(4):
                ks = slice(q4 * 4, (q4 + 1) * 4)
                nc.sync.dma_start(xt[:, ks], x4[tch, :, ks])
            return xt

        def gen_projqk(h, tch, xt, skipchk):
            """q/k projection for one 512-token chunk; yields per k-tile."""
            tsl = slice(tch * IC, (tch + 1) * IC)
            ps_q = psum.tile([P, IC], F32, tag="pa", name="pa")
            ps_k = psum.tile([P, IC], F32, tag="pb", name="pb")
            for kt in range(KD):
                nc.tensor.matmul(ps_q, w_sb["q"][:, kt, h], xt[:, kt],
                                 start=(kt == 0), stop=(kt == KD - 1),
                                 skip_group_check=skipchk)
                nc.tensor.matmul(ps_k, w_sb["k"][:, kt, h], xt[:, kt],
                                 start=(kt == 0), stop=(kt == KD - 1),
                                 skip_group_check=skipchk)
                yield
            nc.vector.tensor_copy(qT[h][:, tsl], ps_q)
            nc.vector.tensor_copy(kT[h][:, tsl], ps_k)

        def gen_projv(h, tch, xt, vtag, skipchk):
            """v projection + v transposes; yields per k-tile."""
            tsl = slice(tch * IC, (tch + 1) * IC)
            ps_v = psum.tile([P, IC], F32, tag=vtag, name=vtag)
            for kt in range(KD):
                nc.tensor.matmul(ps_v, w_sb["v"][:, kt, h], xt[:, kt],
                                 start=(kt == 0), stop=(kt == KD - 1),
                                 skip_group_check=skipchk)
                yield
            nc.vector.tensor_copy(vT[h][:, tsl], ps_v)
            for i in range(IC // P):
                tt0 = tch * (IC // P) + i   # global token tile 0..31
                g = st.tpg
                st.tpg += 1
                nc.tensor.transpose(
                    tp[:, g % 2], vT[h][:, tt0 * P:(tt0 + 1) * P], ident)
                nc.vector.tensor_copy(v_sb[h][:, tt0], tp[:, g % 2])

        def chain2(g1, g2):
            yield from g1
            yield from g2

        def gen_attn(h, b, ic, srot, ostag):
            """Attention for one 512-query chunk of head h.
            S/exp issued LA tiles ahead; yields after each PV.
            Normalize (recip+mult) is deferred via st.norm_q."""
            isl = slice(b * T + ic * IC, b * T + (ic + 1) * IC)
            nj = ic * 4 + 4          # causal: j tiles 0..nj-1
            ps_o = psum.tile([P, IC], F32, tag=ostag, name=ostag)
            pt_acc = smalls.tile([P, IC], BF16, tag="ptacc")

            def s_exp(j):
                m = j - ic * 4
                lo = max(m, 0) * P   # cols < lo fully masked
                tag = srot[j % len(srot)]
                ps_s = psum.tile([P, IC], F32, tag=tag, name=tag)
                nc.tensor.matmul(
                    ps_s[:, lo:],
                    kT[h][:, b * T + j * P: b * T + (j + 1) * P],
                    qT[h][:, b * T + ic * IC + lo: b * T + (ic + 1) * IC],
                    start=True, stop=(m < 0), skip_group_check=True,
                )
                if m >= 0:
                    nc.tensor.matmul(
                        ps_s[:, lo:lo + P], triC, ident,
                        start=False, stop=True, skip_group_check=True,
                    )
                pt = ptpool.tile([P, IC], BF16, tag="pt", name="pt")
                nc.scalar.activation(
                    pt[:, lo:], ps_s[:, lo:],
                    mybir.ActivationFunctionType.Exp, scale=scale)
                return pt, lo

            pts = [s_exp(j) for j in range(min(LA, nj))]
            for j in range(nj):
                if j + LA < nj:
                    pts.append(s_exp(j + LA))
                pt, lo = pts[j]
                nc.tensor.matmul(
                    ps_o[:, lo:], v_sb[h][:, b * TJ + j], pt[:, lo:],
                    start=(j == 0), stop=(j == nj - 1),
                    skip_group_check=True,
                )
                # denominator accumulation, off the PE critical path
                if j == 0:
                    nc.vector.tensor_copy(pt_acc, pt)
                else:
                    nc.vector.tensor_tensor(
                        pt_acc[:, lo:], pt_acc[:, lo:], pt[:, lo:],
                        mybir.AluOpType.add)
                yield
            # free ps_o fast with an unnormalized copy; the denominator +
            # normalize run one chunk later (closure) so nothing here sits
            # on the PE/DVE critical path
            o_u = smalls.tile([P, IC], BF16, tag="ou")
            nc.vector.tensor_copy(o_u, ps_o)

            def norm():
                # den halves: PE ones-matmul -> tiny copy -> gpsimd
                # broadcast -> DVE reciprocal + multiply
                for hf in range(2):
                    hs = slice(hf * (IC // 2), (hf + 1) * (IC // 2))
                    os_ = slice(b * T + ic * IC + hf * (IC // 2),
                                b * T + ic * IC + (hf + 1) * (IC // 2))
                    dn = psum.tile([1, IC // 2], F32, tag="dn", name="dn")
                    nc.tensor.matmul(dn, ones_col, pt_acc[:, hs],
                                     start=True, stop=True,
                                     skip_group_check=True)
                    den_sb = smalls.tile([1, IC // 2], F32, tag="densb")
                    nc.vector.tensor_copy(den_sb, dn)
                    bch = smalls.tile([P, IC // 2], F32, tag="bch")
                    nc.gpsimd.partition_broadcast(bch, den_sb)
                    rbh = smalls.tile([P, IC // 2], F32, tag="rbh")
                    nc.vector.reciprocal_approx_fast(out=rbh, in_=bch)
                    nc.vector.tensor_tensor(
                        outT[h][:, os_], o_u[:, hs], rbh,
                        mybir.AluOpType.mult)

            st.norm_q.append(norm)

        def gen_outproj(b, ic):
            """y tiles (all 2048 out-channels) for one 512-token range.
            Yields after each y tile (2 matmuls)."""
            t0 = (b * T + ic * IC) // P
            for mc in range(D_MODEL // IC):
                msl = slice(mc * IC, (mc + 1) * IC)
                for tl in range(IC // P):
                    tt = t0 + tl
                    yc = st.yc
                    st.yc += 1
                    ps_y = psum.tile([P, IC], F32, tag=f"y{yc % 2}",
                                     name=f"y{yc % 2}")
                    for h in range(HPC):
                        nc.tensor.matmul(
                            ps_y, outT[h][:, tt * P:(tt + 1) * P],
                            wo_sb[:, h, msl],
                            start=(h == 0), stop=(h == HPC - 1),
                            skip_group_check=True,
                        )
                    y_sb = ypool.tile([P, IC], F16, tag="y", name="y")
                    if yc % 2:
                        nc.scalar.copy(y_sb, ps_y)
                    else:
                        nc.vector.tensor_copy(y_sb, ps_y)
                    nc.sync.dma_start(y[tt * P:(tt + 1) * P, msl], y_sb)
                    yield

        # ---- phase A: head-0 projections ----
        # v-pass of tch t (bank y0) interleaves with q/k-pass of tch t+1
        gv_prev = None
        for tch in range(TCH):
            xt = new_xt(tch)
            gqk = gen_projqk(0, tch, xt, False)
            if gv_prev is not None:
                _drive(gqk, KD, gv_prev, KD)
            else:
                _run_gen(gqk)
            gv_prev = gen_projv(0, tch, xt, "y0", False)
            if tch == 0:
                # head-1 weights: behind tch0's x stream, before phase B
                for nm in ("q", "k", "v"):
                    nc.sync.dma_start(w_sb[nm][:, :, 1], w4[nm][1])
        _run_gen(gv_prev)

        # wo needed from phase C; queue its DMA behind the x stream
        wo_sb = wpool.tile([P, HPC, D_MODEL], BF16, tag="wo", name="wo")
        nc.sync.dma_start(wo_sb, woT3)

        # ---- phase B: head-1 projections x head-0 attention ----
        chunks = [(bb, ii) for bb in range(B) for ii in range(NI)]
        for tch in range(TCH):
            bb, ii = chunks[tch]
            xt = new_xt(tch)
            gp = chain2(gen_projqk(1, tch, xt, True),
                        gen_projv(1, tch, xt, "pa", True))
            _drive(gp, 2 * KD,
                   gen_attn(0, bb, ii, ("y0", "y1", "t1"), "t0"), ii * 4 + 4)
            flush_norms(keep=1)

        # ---- phase C: head-1 attention x out-projection (2 chunks back) ----
        pending = []
        for bb, ii in chunks:
            ga = gen_attn(1, bb, ii, ("pa", "pb", "t0"), "t1")
            if len(pending) >= 2:
                pb, pi = pending.pop(0)
                _drive(gen_outproj(pb, pi), 16, ga, ii * 4 + 4)
            else:
                _run_gen(ga)
            pending.append((bb, ii))
            flush_norms(keep=1)
        flush_norms(keep=0)
        for pb, pi in pending:
            _run_gen(gen_outproj(pb, pi))


def _build(cfg=None):
    nc = bacc.Bacc("TRN2", target_bir_lowering=False, debug=False,
                   num_devices=NCORES)
    xT = nc.dram_tensor("xT", [TCH * P, KD * IC], BF16,
                        kind="ExternalInput").ap()
    wqT = nc.dram_tensor("wqT", [HPC * P, KD * D], BF16,
                         kind="ExternalInput").ap()
    wkT = nc.dram_tensor("wkT", [HPC * P, KD * D], BF16,
                         kind="ExternalInput").ap()
    wvT = nc.dram_tensor("wvT", [HPC * P, KD * D], BF16,
                         kind="ExternalInput").ap()
    woT = nc.dram_tensor("woT", [HPC * D, D_MODEL], BF16,
                         kind="ExternalInput").ap()
    y = nc.dram_tensor("y", [BT, D_MODEL], F16, kind="ExternalOutput").ap()
    with tile.TileContext(nc) as tc:
        _emit(tc, xT, wqT, wkT, wvT, woT, y)
    nc.compile()
    return nc


def _prep_inputs(x, Wq, Wk, Wv, Wo, cfg=None):
    bf = ml_dtypes.bfloat16
    xT = np.ascontiguousarray(
        np.asarray(x, np.float32).reshape(BT, D_MODEL).T).astype(bf)
    # x4[tch, p, kt, i] = xT[kt*P + p, tch*IC + i], flattened 2D
    x4 = np.ascontiguousarray(
        xT.reshape(KD, P, TCH, IC).transpose(2, 1, 0, 3)
    ).reshape(TCH * P, KD * IC)
    in_maps = []
    for c in range(NCORES):
        rows = slice(c * HPC * D, (c + 1) * HPC * D)

        def wprep(W):
            # w4[h, p, kt, d] = W[rows][h*D + d, kt*P + p], flattened 2D
            wT = np.ascontiguousarray(np.asarray(W)[rows].T).astype(bf)
            return np.ascontiguousarray(
                wT.reshape(KD, P, HPC, D).transpose(2, 1, 0, 3)
            ).reshape(HPC * P, KD * D)

        in_maps.append({
            "xT": x4,
            "wqT": wprep(Wq),
            "wkT": wprep(Wk),
            "wvT": wprep(Wv),
            "woT": np.ascontiguousarray(
                np.asarray(Wo)[:, rows].T).astype(bf),
        })
    return in_maps


def run(x, Wq, Wk, Wv, Wo, cfg=None, trace=False):
    nc = _build(cfg)
    in_maps = _prep_inputs(x, Wq, Wk, Wv, Wo, cfg)
    try:
        res = run_bass_kernel_spmd(nc, in_maps, core_ids=list(range(NCORES)),
                                   trace=trace)
    except Exception:
        res = run_bass_kernel_spmd(nc, in_maps, core_ids=list(range(NCORES)),
                                   trace=trace)
    y = np.zeros((BT, D_MODEL), np.float32)
    for r in res.results:
        y += np.asarray(r["y"], np.float32)
    return y.reshape(B, T, D_MODEL), res


def kernel(x, Wq, Wk, Wv, Wo):
    y, _ = run(x, Wq, Wk, Wv, Wo)
    return y
